# revision 1
# baseline (speedup 1.0000x reference)
"""Trainium2 Bass kernel for nn_CrossAttention (B=2, T=2048, D=1024, H=16, hd=64).

Sharding: 32 (batch, head) units over 8 cores -> each core handles 1 batch and
4 contiguous heads (core c: batch c//4, heads (c%4)*4 .. +4).  Per-core kernel
computes the partial c_proj output for its 4 heads; host sums the 4 partials
per batch and adds bc.

Per-core dataflow (all activations kept transposed, D-on-partitions):
  qpT [256, 2048] = WqT.T @ qT   (+bq)      kpT likewise
  per head h: S.T[tv, tq] = khT.T-slice matmuls (K=64)
              expS = exp(S.T / 8)           (ScalarE, scale fused)
              y_ext[65, 2048] += [vh|1].T @ expS   (ones row -> colsum)
              yT = y_ext[:64] * (1/colsum) (recip + DMA partition-broadcast)
  out_partial[2048, 1024] = yallT.T @ WcT  (K=256)

All matmuls run as float32r (full-rate fp32 PE mode).
"""

import sys

sys.path.insert(0, "/opt/trn_rl_repo")

import numpy as np

import concourse.bacc as bacc
import concourse.bass as bass
import concourse.mybir as mybir
import concourse.tile as tile
from concourse.bass_utils import run_bass_kernel_spmd

F32 = mybir.dt.float32
F32R = mybir.dt.float32r

T = 2048          # sequence length (both q and kv)
D = 1024          # model dim
HL = 4            # heads per core
HD = 64           # head dim
DH = HL * HD      # 256 local projected dim
P = 128
JT_G = DH // P  # 2
SCALE = 1.0 / 8.0  # 1/sqrt(64)

N_CORES = 8

_cache = {}


def r(ap):
    return ap.bitcast(F32R)


def build_nc():
    if "nc" in _cache:
        return _cache["nc"]
    nc = bacc.Bacc(
        "TRN2",
        target_bir_lowering=False,
        debug=False,
        num_devices=N_CORES,
    )

    qT = nc.declare_dram_parameter("qT", [D, T], F32R, isOutput=False)
    kT = nc.declare_dram_parameter("kT", [D, T], F32R, isOutput=False)
    v_sl = nc.declare_dram_parameter("v_sl", [T, DH], F32R, isOutput=False)
    WqT = nc.declare_dram_parameter("WqT", [D, DH], F32R, isOutput=False)
    WkT = nc.declare_dram_parameter("WkT", [D, DH], F32R, isOutput=False)
    WcT = nc.declare_dram_parameter("WcT", [DH, D], F32R, isOutput=False)
    bqk = nc.declare_dram_parameter("bqk", [P, 4], F32, isOutput=False)
    ones = nc.declare_dram_parameter("ones", [P, T // P], F32R, isOutput=False)
    out = nc.declare_dram_parameter("out", [T, D], F32, isOutput=True)
    out2 = nc.declare_dram_parameter("out2", [T, D], F32, isOutput=True)
    debug = bool(int(__import__("os").environ.get("BASSDBG", "0")))
    if debug:
        dbg_qpT = nc.declare_dram_parameter("dbg_qpT", [P, JT_G, T], F32R, isOutput=True)
        dbg_kpT = nc.declare_dram_parameter("dbg_kpT", [P, JT_G, T], F32R, isOutput=True)
        dbg_es = nc.declare_dram_parameter("dbg_es", [P, 1024], F32R, isOutput=True)
        dbg_yall = nc.declare_dram_parameter("dbg_yall", [P, JT_G, T], F32R, isOutput=True)
        dbg_rec = nc.declare_dram_parameter("dbg_rec", [HD, T // 2], F32, isOutput=True)
        dbg_col = nc.declare_dram_parameter("dbg_col", [1, T // 2], F32, isOutput=True)
        dbg_ve = nc.declare_dram_parameter("dbg_ve", [P, T // P, HD + 1], F32R, isOutput=True)

    KT = D // P   # 8 din tiles
    JT = DH // P  # 2 dout tiles

    with tile.TileContext(nc) as tc:
        with (
            tc.tile_pool(name="wpool", bufs=1) as wpool,
            tc.tile_pool(name="stream", bufs=8) as stream,
            tc.tile_pool(name="projsb", bufs=1) as projsb,
            tc.tile_pool(name="vpool", bufs=1) as vpool,
            tc.tile_pool(name="epool", bufs=4) as epool,
            tc.tile_pool(name="npool", bufs=2) as npool,
            tc.tile_pool(name="opool", bufs=3) as opool,
            tc.tile_pool(name="psA", bufs=2, space="PSUM") as psA,
            tc.tile_pool(name="psB", bufs=2, space="PSUM") as psB,
            tc.tile_pool(name="drampool", bufs=2, space="DRAM") as drampool,
        ):
            # ---- weights / constants ----
            wq_sb = wpool.tile([P, KT, DH], F32R, name="wq_sb")
            nc.sync.dma_start(wq_sb[:], WqT.ap().rearrange("(a p) m -> p a m", p=P))
            wk_sb = wpool.tile([P, KT, DH], F32R, name="wk_sb")
            nc.sync.dma_start(wk_sb[:], WkT.ap().rearrange("(a p) m -> p a m", p=P))
            wc_sb = wpool.tile([P, JT, D], F32R, name="wc_sb")
            nc.sync.dma_start(wc_sb[:], WcT.ap().rearrange("(a p) m -> p a m", p=P))
            bias_sb = wpool.tile([P, 4], F32, name="bias_sb")  # [bq0,bq1,bk0,bk1]
            nc.sync.dma_start(bias_sb[:], bqk.ap())

            # ---- v_ext tiles: [v_h | ones] per head ----
            v_re = v_sl.ap().rearrange("(t p) d -> p t d", p=P)  # [128, 16, 256]
            vext = []
            for h in range(HL):
                ve = vpool.tile([P, T // P, HD + 1], F32R, name=f"vext{h}")
                nc.sync.dma_start(ve[:, :, 0:HD], v_re[:, :, h * HD:(h + 1) * HD])
                nc.sync.dma_start(
                    ve[:, :, HD:HD + 1],
                    ones.ap().unsqueeze(2),
                )
                vext.append(ve)

            # ---- projections: xpT[j*128+p, t], one T column-group ----
            def project_cp(xT_dram, w_sb, bias_col0, name, xpT, cp):
                xt_tiles = []
                for i in range(KT):
                    xt = stream.tile([P, 1024], F32R, tag="xt",
                                     name=f"{name}t{cp}_{i}")
                    nc.sync.dma_start(
                        xt[:],
                        xT_dram.ap()[i * P:(i + 1) * P,
                                     cp * 1024:(cp + 1) * 1024],
                    )
                    xt_tiles.append(xt)
                groups = [
                    psA.tile([P, 1024], F32, tag="psA", name=f"{name}p{j}{cp}")
                    for j in range(JT)
                ]
                for i in range(KT):
                    for j in range(JT):
                        for c in range(2):
                            nc.tensor.matmul(
                                groups[j][:, c * 512:(c + 1) * 512],
                                w_sb[:, i, j * P:(j + 1) * P],
                                xt_tiles[i][:, c * 512:(c + 1) * 512],
                                start=(i == 0),
                                stop=(i == KT - 1),
                            )
                for j in range(JT):
                    nc.vector.tensor_tensor(
                        xpT[:, j, cp * 1024:(cp + 1) * 1024],
                        groups[j][:],
                        bias_sb[:, bias_col0 + j:bias_col0 + j + 1]
                        .to_broadcast((P, 1024)),
                        mybir.AluOpType.add,
                    )

            kpT = projsb.tile([P, JT, T], F32R, name="kpT")
            qpT = projsb.tile([P, JT, T], F32R, name="qpT")
            project_cp(kT, wk_sb, 2, "k", kpT, 0)
            project_cp(kT, wk_sb, 2, "k", kpT, 1)
            project_cp(qT, wq_sb, 0, "q", qpT, 0)

            # ---- attention per (head, Tq-half) pass ----
            HT = T // 2  # 1024
            yallT = projsb.tile([P, JT, T], F32R, name="yallT")

            def attn_pass(h, half):
                j = h // 2
                p0 = (h % 2) * HD
                khT = kpT[p0:p0 + HD, j, :]   # [64, 2048]
                qhT = qpT[p0:p0 + HD, j, :]
                y_ps = psB.tile([HD + 1, HT], F32, tag="psB", name=f"y{h}_{half}")
                for mv in range(T // P):
                    s_ps = psA.tile([P, HT], F32, tag="psA",
                                    name=f"s{h}_{mv}_{half}")
                    for c in range(2):
                        q0 = half * HT + c * 512
                        nc.tensor.matmul(
                            s_ps[:, c * 512:(c + 1) * 512],
                            khT[:, mv * P:(mv + 1) * P],
                            qhT[:, q0:q0 + 512],
                            start=True,
                            stop=True,
                        )
                    es = epool.tile([P, HT], F32R, tag="es",
                                    name=f"e{h}_{mv}_{half}")
                    nc.scalar.activation(
                        es[:], s_ps[:], mybir.ActivationFunctionType.Exp,
                        scale=SCALE,
                    )
                    if debug and h == 0 and mv == 0 and half == 0:
                        nc.sync.dma_start(dbg_es.ap(), es[:])
                    for c in range(2):
                        nc.tensor.matmul(
                            y_ps[:, c * 512:(c + 1) * 512],
                            vext[h][:, mv, :],
                            es[:, c * 512:(c + 1) * 512],
                            start=(mv == 0),
                            stop=(mv == T // P - 1),
                        )
                # normalize: yT = y_ps[:64] * 1/colsum into yallT half
                bcast = npool.tile([HD + 1, HT], F32, tag="bcast",
                                   name=f"bc{h}_{half}")
                nc.vector.tensor_copy(bcast[HD:HD + 1, :], y_ps[HD:HD + 1, :])
                if debug and h == 0 and half == 0:
                    nc.sync.dma_start(dbg_col.ap(), bcast[HD:HD + 1, :])
                nc.vector.reciprocal(bcast[HD:HD + 1, :], bcast[HD:HD + 1, :])
                dscr = drampool.tile([1, HT], F32, tag="dscr",
                                     name=f"dscr{h}_{half}")
                nc.sync.dma_start(dscr[:], bcast[HD:HD + 1, :])
                nc.sync.dma_start(bcast[0:HD, :], dscr[:].to_broadcast((HD, HT)))
                ynorm = npool.tile([HD, HT], F32R, tag="ynorm",
                                   name=f"yn{h}_{half}")
                nc.vector.tensor_tensor(
                    ynorm[:], y_ps[0:HD, :], bcast[0:HD, :],
                    mybir.AluOpType.mult
                )
                nc.sync.dma_start(
                    yallT[p0:p0 + HD, j, half * HT:(half + 1) * HT], ynorm[:]
                )
                if debug and h == 0 and half == 0:
                    nc.sync.dma_start(dbg_rec.ap(), bcast[0:HD, :])

            def cproj_part(j, out_dram, mts, copy_engine):
                # out_j[t, :] = yallT[:, j, t].T @ wc[j]  (K=128, no accum)
                for mt in mts:
                    o_ps = psA.tile([P, 1024], F32, tag="psA", name=f"o{j}_{mt}")
                    for nch in range(2):
                        nc.tensor.matmul(
                            o_ps[:, nch * 512:(nch + 1) * 512],
                            yallT[:, j, mt * P:(mt + 1) * P],
                            wc_sb[:, j, nch * 512:(nch + 1) * 512],
                            start=True,
                            stop=True,
                        )
                    o_sb = opool.tile([P, 1024], F32, tag="osb", name=f"ot{j}_{mt}")
                    if copy_engine == "act":
                        nc.scalar.copy(o_sb[:], o_ps[:])
                    else:
                        nc.vector.tensor_copy(o_sb[:], o_ps[:])
                    nc.sync.dma_start(out_dram.ap()[mt * P:(mt + 1) * P, :], o_sb[:])

            project_cp(qT, wq_sb, 0, "q", qpT, 1)
            for h in range(HL):
                for half in range(2):
                    attn_pass(h, half)
            if debug:
                nc.sync.dma_start(dbg_yall.ap(), yallT[:])
            cproj_part(0, out, [m for m in range(16) if m % 2 == 0], "act")
            cproj_part(0, out, [m for m in range(16) if m % 2 == 1], "dve")
            cproj_part(1, out2, [m for m in range(16) if m % 2 == 0], "act")
            cproj_part(1, out2, [m for m in range(16) if m % 2 == 1], "dve")

    nc.compile()
    _cache["nc"] = nc
    return nc


def make_in_maps(k, q, v, Wk, bk, Wq, bq, Wc, bc):
    k = np.asarray(k, dtype=np.float32)
    q = np.asarray(q, dtype=np.float32)
    v = np.asarray(v, dtype=np.float32)
    Wk = np.asarray(Wk, dtype=np.float32)
    Wq = np.asarray(Wq, dtype=np.float32)
    Wc = np.asarray(Wc, dtype=np.float32)
    bk = np.asarray(bk, dtype=np.float32)
    bq = np.asarray(bq, dtype=np.float32)
    in_maps = []
    for c in range(N_CORES):
        b = c // 4
        h0 = (c % 4) * HL
        sl = slice(h0 * HD, h0 * HD + DH)
        bq_t = np.ascontiguousarray(bq[sl].reshape(2, P).T)  # [128, 2]
        bk_t = np.ascontiguousarray(bk[sl].reshape(2, P).T)
        bqk = np.concatenate([bq_t, bk_t], axis=1)           # [128, 4]
        in_maps.append({
            "qT": np.ascontiguousarray(q[b].T),
            "kT": np.ascontiguousarray(k[b].T),
            "v_sl": np.ascontiguousarray(v[b][:, sl]),
            "WqT": np.ascontiguousarray(Wq[sl, :].T),
            "WkT": np.ascontiguousarray(Wk[sl, :].T),
            "WcT": np.ascontiguousarray(Wc[:, sl].T),
            "bqk": np.ascontiguousarray(bqk),
            "ones": np.ones((P, T // P), dtype=np.float32),
        })
    return in_maps


def kernel(k, q, v, Wk, bk, Wq, bq, Wc, bc, _trace=False, _trace_cores=None):
    bc = np.asarray(bc, dtype=np.float32)
    nc = build_nc()
    in_maps = make_in_maps(k, q, v, Wk, bk, Wq, bq, Wc, bc)
    res = run_bass_kernel_spmd(
        nc, in_maps, core_ids=list(range(N_CORES)),
        trace=_trace, trace_cores=_trace_cores,
    )
    outs = [res.results[c]["out"] + res.results[c]["out2"] for c in range(N_CORES)]
    full = np.stack([
        outs[0] + outs[1] + outs[2] + outs[3],
        outs[4] + outs[5] + outs[6] + outs[7],
    ]) + bc[None, None, :]
    kernel.last_result = res
    return full.astype(np.float32)



# revision 3
# speedup vs baseline: 1.3085x; 1.3085x over previous
"""Trainium2 Bass kernel for nn_CrossAttention (B=2, T=2048, D=1024, H=16, hd=64).

Sharding: 32 (batch, head) units over 8 cores -> each core handles 1 batch and
4 contiguous heads (core c: batch c//4, heads (c%4)*4 .. +4), grouped as two
head-pairs j in {0,1}.  Host sums the 4 partial c_proj outputs per batch and
adds bc.

Per-core dataflow (bf16 operands, D-on-partitions activations):
  qpT/kpT [128, 2, 2048] bf16 = W.T @ xT (+b), K=1024 accumulated in PSUM
  attention per (pair j, q-chunk 1024):
    S pair tile [128(kv), 2(head), 1024(q)] f32 PSUM — the two heads' K=64
    score matmuls run concurrently on PE row-groups 0/64
    exp: one ScalarE activation over [128, 2048] (both heads) -> es bf16 SBUF
    av: [v_h | ones].T @ es accumulates y/colsum in [65, 1024] f32 per head
    norm: colsum -> reciprocal_approx_fast -> gpsimd partition_broadcast ->
    DVE multiply -> yallT bf16 (DMA partition-shift for odd heads)
  cproj: out[t,:] = yallT.T @ WcT, K=256 accumulated in PSUM, bf16 out
"""

import sys

sys.path.insert(0, "/opt/trn_rl_repo")

import numpy as np
import ml_dtypes

import concourse.bacc as bacc
import concourse.bass as bass
import concourse.mybir as mybir
import concourse.tile as tile
from concourse.bass_utils import run_bass_kernel_spmd

F32 = mybir.dt.float32
BF16 = mybir.dt.bfloat16

T = 2048          # sequence length (q and kv)
D = 1024          # model dim
HL = 4            # heads per core
HD = 64           # head dim
DH = HL * HD      # 256 local projected dim
P = 128
QC = 1024         # q chunk for attention
NKV = T // P      # 16 kv tiles
SCALE = 1.0 / 8.0  # 1/sqrt(64)

N_CORES = 8

_cache = {}


def build_nc():
    if "nc" in _cache:
        return _cache["nc"]
    nc = bacc.Bacc(
        "TRN2",
        target_bir_lowering=False,
        debug=False,
        num_devices=N_CORES,
    )

    qT = nc.declare_dram_parameter("qT", [D, T], BF16, isOutput=False)
    kT = nc.declare_dram_parameter("kT", [D, T], BF16, isOutput=False)
    vext = nc.declare_dram_parameter(
        "vext", [P, HL, NKV, HD + 1], BF16, isOutput=False
    )
    Wq_r = nc.declare_dram_parameter("Wq_r", [P, 8, DH], BF16, isOutput=False)
    Wk_r = nc.declare_dram_parameter("Wk_r", [P, 8, DH], BF16, isOutput=False)
    Wc_r = nc.declare_dram_parameter("Wc_r", [P, 2, D], BF16, isOutput=False)
    bqk = nc.declare_dram_parameter("bqk", [P, 4], F32, isOutput=False)
    out = nc.declare_dram_parameter("out", [T, D], BF16, isOutput=True)

    with tile.TileContext(nc) as tc:
        with (
            tc.tile_pool(name="wpool", bufs=1) as wpool,
            tc.tile_pool(name="xpool", bufs=8) as xpool,
            tc.tile_pool(name="projsb", bufs=1) as projsb,
            tc.tile_pool(name="espool", bufs=3) as espool,
            tc.tile_pool(name="npool", bufs=2) as npool,
            tc.tile_pool(name="opool", bufs=3) as opool,
            tc.tile_pool(name="ps", bufs=2, space="PSUM") as ps,
        ):
            # ---- weights / constants (host pre-packed, contiguous DMA) ----
            wq_sb = wpool.tile([P, 8, DH], BF16, name="wq_sb")
            nc.sync.dma_start(wq_sb[:], Wq_r.ap())
            wk_sb = wpool.tile([P, 8, DH], BF16, name="wk_sb")
            nc.sync.dma_start(wk_sb[:], Wk_r.ap())
            wc_sb = wpool.tile([P, 2, D], BF16, name="wc_sb")
            nc.sync.dma_start(wc_sb[:], Wc_r.ap())
            bias_sb = wpool.tile([P, 4], F32, name="bias_sb")  # [bq0,bq1,bk0,bk1]
            nc.sync.dma_start(bias_sb[:], bqk.ap())
            ve_sb = wpool.tile([P, HL, NKV, HD + 1], BF16, name="ve_sb")
            nc.sync.dma_start(ve_sb[:], vext.ap())

            # preload the exp activation table during the DMA ramp
            warm_sb = wpool.tile([P, 4], F32, name="warm_sb")
            nc.scalar.activation(
                warm_sb[:], bias_sb[:], mybir.ActivationFunctionType.Exp,
                scale=0.0,
            )

            kpT = projsb.tile([P, 2, T], BF16, name="kpT")
            qpT = projsb.tile([P, 2, T], BF16, name="qpT")
            yallT = projsb.tile([P, 2, T], BF16, name="yallT")

            # ---- projections: xpT[j*128+p, t] = sum_i W[i,j].T @ x[i] ----
            def project(xT_dram, w_sb, bias_col0, xpT, nm):
                xts = []
                for i in range(8):
                    xt = xpool.tile([P, T], BF16, tag="xt", name=f"x{nm}{i}")
                    nc.sync.dma_start(xt[:], xT_dram.ap()[i * P:(i + 1) * P, :])
                    xts.append(xt)
                quads = [
                    ps.tile([P, 4, 512], F32, tag="big", name=f"pq{nm}{j}")
                    for j in range(2)
                ]
                for i in range(8):
                    for j in range(2):
                        for tc_ in range(4):
                            nc.tensor.matmul(
                                quads[j][:, tc_, :],
                                w_sb[:, i, j * P:(j + 1) * P],
                                xts[i][:, tc_ * 512:(tc_ + 1) * 512],
                                start=(i == 0),
                                stop=(i == 7),
                            )
                for j in range(2):
                    nc.vector.tensor_tensor(
                        xpT[:, j, :],
                        quads[j].rearrange("p a b -> p (a b)"),
                        bias_sb[:, bias_col0 + j:bias_col0 + j + 1]
                        .to_broadcast((P, T)),
                        mybir.AluOpType.add,
                    )

            project(kT, wk_sb, 2, kpT, "k")
            project(qT, wq_sb, 0, qpT, "q")

            # ---- attention per (head-pair j, q-chunk c) ----
            def attn_unit(j, c):
                q0 = c * QC
                s_ps = ps.tile([P, 2, QC], F32, tag="big", name=f"s{j}{c}")
                y_ps = ps.tile([P, 2, QC], F32, tag="big", name=f"y{j}{c}")
                es_tiles = {}

                def s_mm(kk):
                    for s in range(2):  # head slot: partitions s*64..s*64+64
                        p0 = s * 64
                        for nch in range(2):
                            nc.tensor.matmul(
                                s_ps[:, s, nch * 512:(nch + 1) * 512],
                                kpT[p0:p0 + HD, j, kk * P:(kk + 1) * P],
                                qpT[p0:p0 + HD, j,
                                    q0 + nch * 512:q0 + (nch + 1) * 512],
                                start=True,
                                stop=True,
                            )

                def av_mm(kk):
                    es = es_tiles.pop(kk)
                    for s in range(2):
                        h = 2 * j + s
                        for nch in range(2):
                            nc.tensor.matmul(
                                y_ps[0:HD + 1, s, nch * 512:(nch + 1) * 512],
                                ve_sb[:, h, kk, :],
                                es[:, s, nch * 512:(nch + 1) * 512],
                                start=(kk == 0),
                                stop=(kk == NKV - 1),
                            )

                # av lags exp by 2 kv-steps so the y-slot WAR wait (previous
                # unit's normalize) never blocks this unit's S/exp stream.
                for kk in range(NKV):
                    s_mm(kk)
                    es = espool.tile([P, 2, QC], BF16, tag="es",
                                     name=f"e{j}{c}{kk}")
                    nc.scalar.activation(
                        es[:], s_ps[:], mybir.ActivationFunctionType.Exp,
                        scale=SCALE,
                    )
                    es_tiles[kk] = es
                    if kk >= 2:
                        av_mm(kk - 2)
                av_mm(NKV - 2)
                av_mm(NKV - 1)

                # normalize: yallT[.., q0:q0+QC] = y/colsum per head
                for s in range(2):
                    p0 = s * 64
                    colsum = npool.tile([1, QC], F32, tag="colsum",
                                        name=f"cs{j}{c}{s}")
                    nc.vector.tensor_copy(colsum[:], y_ps[HD:HD + 1, s, :])
                    recip = npool.tile([1, QC], F32, tag="recip",
                                       name=f"rc{j}{c}{s}")
                    nc.vector.reciprocal_approx_fast(out=recip[:], in_=colsum[:])
                    bcast = npool.tile([HD, QC], F32, tag="bcast",
                                       name=f"bc{j}{c}{s}")
                    nc.gpsimd.partition_broadcast(bcast[:], recip[:])
                    if p0 == 0:
                        nc.vector.tensor_tensor(
                            yallT[0:HD, j, q0:q0 + QC],
                            y_ps[0:HD, s, :], bcast[:],
                            mybir.AluOpType.mult,
                        )
                    else:
                        yn = npool.tile([HD, QC], BF16, tag="yn",
                                        name=f"yn{j}{c}{s}")
                        nc.vector.tensor_tensor(
                            yn[:], y_ps[0:HD, s, :], bcast[:],
                            mybir.AluOpType.mult,
                        )
                        nc.sync.dma_start(
                            yallT[p0:p0 + HD, j, q0:q0 + QC], yn[:]
                        )

            for c in range(T // QC):
                for j in range(2):
                    attn_unit(j, c)

            # ---- c_proj: out[t,:] = sum_j yallT[:,j,t].T @ wc[j] ----
            for tg in range(8):  # two t-tiles per psum alloc
                cp = ps.tile([P, 2, 2, 512], F32, tag="big", name=f"cp{tg}")
                for t2 in range(2):
                    tt = tg * 2 + t2
                    for j in range(2):
                        for nch in range(2):
                            nc.tensor.matmul(
                                cp[:, t2, nch, :],
                                yallT[:, j, tt * P:(tt + 1) * P],
                                wc_sb[:, j, nch * 512:(nch + 1) * 512],
                                start=(j == 0),
                                stop=(j == 1),
                            )
                for t2 in range(2):
                    tt = tg * 2 + t2
                    o_sb = opool.tile([P, D], BF16, tag="osb", name=f"o{tt}")
                    nc.vector.tensor_copy(
                        o_sb[:], cp[:, t2, :, :].rearrange("p a b -> p (a b)")
                    )
                    nc.sync.dma_start(out.ap()[tt * P:(tt + 1) * P, :], o_sb[:])

    nc.compile()
    _cache["nc"] = nc
    return nc


def make_in_maps(k, q, v, Wk, bk, Wq, bq, Wc, bc):
    bf = ml_dtypes.bfloat16
    k = np.asarray(k, dtype=np.float32)
    q = np.asarray(q, dtype=np.float32)
    v = np.asarray(v, dtype=np.float32)
    Wk = np.asarray(Wk, dtype=np.float32)
    Wq = np.asarray(Wq, dtype=np.float32)
    Wc = np.asarray(Wc, dtype=np.float32)
    bk = np.asarray(bk, dtype=np.float32)
    bq = np.asarray(bq, dtype=np.float32)
    in_maps = []
    for cidx in range(N_CORES):
        b = cidx // 4
        h0 = (cidx % 4) * HL
        sl = slice(h0 * HD, h0 * HD + DH)
        bq_t = np.ascontiguousarray(bq[sl].reshape(2, P).T)  # [128, 2]
        bk_t = np.ascontiguousarray(bk[sl].reshape(2, P).T)
        bqk = np.concatenate([bq_t, bk_t], axis=1)           # [128, 4]
        # vext [P, HL, NKV, HD+1]: [p, h, m, d] = v[m*128+p, sl][h*64+d],
        # ones at d=64 (colsum row for the av matmul)
        vsl = v[b][:, sl]                                    # [T, 256]
        ve = np.ones((P, HL, NKV, HD + 1), dtype=np.float32)
        ve[:, :, :, 0:HD] = (
            vsl.reshape(NKV, P, HL, HD).transpose(1, 2, 0, 3)
        )
        # Wq_r [128, 8, 256]: [p, i, m] = Wq[sl,:].T[i*128+p, m]
        wq_t = Wq[sl, :].T.reshape(8, P, DH).transpose(1, 0, 2)
        wk_t = Wk[sl, :].T.reshape(8, P, DH).transpose(1, 0, 2)
        wc_t = Wc[:, sl].T.reshape(2, P, D).transpose(1, 0, 2)
        in_maps.append({
            "qT": np.ascontiguousarray(q[b].T).astype(bf),
            "kT": np.ascontiguousarray(k[b].T).astype(bf),
            "vext": np.ascontiguousarray(ve).astype(bf),
            "Wq_r": np.ascontiguousarray(wq_t).astype(bf),
            "Wk_r": np.ascontiguousarray(wk_t).astype(bf),
            "Wc_r": np.ascontiguousarray(wc_t).astype(bf),
            "bqk": np.ascontiguousarray(bqk),
        })
    return in_maps


def kernel(k, q, v, Wk, bk, Wq, bq, Wc, bc, _trace=False, _trace_cores=None):
    bc = np.asarray(bc, dtype=np.float32)
    nc = build_nc()
    in_maps = make_in_maps(k, q, v, Wk, bk, Wq, bq, Wc, bc)
    res = run_bass_kernel_spmd(
        nc, in_maps, core_ids=list(range(N_CORES)),
        trace=_trace, trace_cores=_trace_cores,
    )
    outs = [res.results[c]["out"].astype(np.float32) for c in range(N_CORES)]
    full = np.stack([
        outs[0] + outs[1] + outs[2] + outs[3],
        outs[4] + outs[5] + outs[6] + outs[7],
    ]) + bc[None, None, :]
    kernel.last_result = res
    return full.astype(np.float32)


# revision 8
# speedup vs baseline: 1.3734x; 1.0496x over previous
"""Trainium2 Bass kernel for nn_CrossAttention (B=2, T=2048, D=1024, H=16, hd=64).

Sharding: 32 (batch, head) units over 8 cores -> each core handles 1 batch and
4 contiguous heads (core c: batch c//4, heads (c%4)*4 .. +4), grouped as two
head-pairs j in {0,1}.  Host sums the 4 partial c_proj outputs per batch and
adds bc.

Per-core dataflow (bf16 operands, D-on-partitions activations):
  qpT/kpT [128, 2, 2048] bf16 = W.T @ xT (+b), K=1024 accumulated in PSUM
  attention per (pair j, q-chunk 1024):
    S pair tile [128(kv), 2(head), 1024(q)] f32 PSUM — the two heads' K=64
    score matmuls run concurrently on PE row-groups 0/64
    exp: one ScalarE activation over [128, 2048] (both heads) -> es bf16 SBUF
    av: [v_h | ones].T @ es accumulates y/colsum in [65, 1024] f32 per head
    norm: colsum -> reciprocal_approx_fast -> gpsimd partition_broadcast ->
    DVE multiply -> yallT bf16 (DMA partition-shift for odd heads)
  cproj: out[t,:] = yallT.T @ WcT, K=256 accumulated in PSUM, bf16 out
"""

import sys

sys.path.insert(0, "/opt/trn_rl_repo")

import numpy as np
import ml_dtypes

import concourse.bacc as bacc
import concourse.bass as bass
import concourse.mybir as mybir
import concourse.tile as tile
from concourse.bass_utils import run_bass_kernel_spmd

F32 = mybir.dt.float32
BF16 = mybir.dt.bfloat16

T = 2048          # sequence length (q and kv)
D = 1024          # model dim
HL = 4            # heads per core
HD = 64           # head dim
DH = HL * HD      # 256 local projected dim
P = 128
QC = 1024         # q chunk for attention
NKV = T // P      # 16 kv tiles
SCALE = 1.0 / 8.0  # 1/sqrt(64)

N_CORES = 8

_cache = {}


def build_nc():
    if "nc" in _cache:
        return _cache["nc"]
    nc = bacc.Bacc(
        "TRN2",
        target_bir_lowering=False,
        debug=False,
        num_devices=N_CORES,
    )

    qT = nc.declare_dram_parameter("qT", [D, T], BF16, isOutput=False)
    kT = nc.declare_dram_parameter("kT", [D, T], BF16, isOutput=False)
    vext = nc.declare_dram_parameter(
        "vext", [P, HL, NKV, HD + 1], BF16, isOutput=False
    )
    Wq_r = nc.declare_dram_parameter("Wq_r", [P, 8, DH], BF16, isOutput=False)
    Wk_r = nc.declare_dram_parameter("Wk_r", [P, 8, DH], BF16, isOutput=False)
    Wc_r = nc.declare_dram_parameter("Wc_r", [P, 2, D], BF16, isOutput=False)
    bqk = nc.declare_dram_parameter("bqk", [P, 4], F32, isOutput=False)
    out = nc.declare_dram_parameter("out", [T, D], BF16, isOutput=True)

    with tile.TileContext(nc) as tc:
        with (
            tc.tile_pool(name="wpool", bufs=1) as wpool,
            tc.tile_pool(name="xpool", bufs=8) as xpool,
            tc.tile_pool(name="projsb", bufs=1) as projsb,
            tc.tile_pool(name="espool", bufs=4) as espool,
            tc.tile_pool(name="npool", bufs=2) as npool,
            tc.tile_pool(name="opool", bufs=3) as opool,
            tc.tile_pool(name="ps", bufs=2, space="PSUM") as ps,
        ):
            # ---- weights / constants (host pre-packed, contiguous DMA).
            # Order by first use: bias+proj weights, then k/q stream in
            # project(); wc/vext (needed much later) are deferred below.
            bias_sb = wpool.tile([P, 4], F32, name="bias_sb")  # [bq0,bq1,bk0,bk1]
            nc.sync.dma_start(bias_sb[:], bqk.ap())
            wk_sb = wpool.tile([P, 8, DH], BF16, name="wk_sb")
            nc.sync.dma_start(wk_sb[:], Wk_r.ap())
            wq_sb = wpool.tile([P, 8, DH], BF16, name="wq_sb")
            nc.sync.dma_start(wq_sb[:], Wq_r.ap())
            wc_sb = wpool.tile([P, 2, D], BF16, name="wc_sb")
            ve_sb = wpool.tile([P, HL, NKV, HD + 1], BF16, name="ve_sb")

            # preload the exp activation table during the DMA ramp
            warm_sb = wpool.tile([P, 4], F32, name="warm_sb")
            nc.scalar.activation(
                warm_sb[:], bias_sb[:], mybir.ActivationFunctionType.Exp,
                scale=0.0,
            )

            kpT = projsb.tile([P, 2, T], BF16, name="kpT")
            qpT = projsb.tile([P, 2, T], BF16, name="qpT")
            yallT = projsb.tile([P, 2, T], BF16, name="yallT")

            # ---- projections: xpT[j*128+p, t] = sum_i W[i,j].T @ x[i] ----
            def project(xT_dram, w_sb, bias_col0, xpT, nm):
                xts = []
                for i in range(8):
                    xt = xpool.tile([P, T], BF16, tag="xt", name=f"x{nm}{i}")
                    nc.sync.dma_start(xt[:], xT_dram.ap()[i * P:(i + 1) * P, :])
                    xts.append(xt)
                quads = [
                    ps.tile([P, 4, 512], F32, tag="big", name=f"pq{nm}{j}")
                    for j in range(2)
                ]
                for i in range(8):
                    for j in range(2):
                        for tc_ in range(4):
                            nc.tensor.matmul(
                                quads[j][:, tc_, :],
                                w_sb[:, i, j * P:(j + 1) * P],
                                xts[i][:, tc_ * 512:(tc_ + 1) * 512],
                                start=(i == 0),
                                stop=(i == 7),
                            )
                for j in range(2):
                    nc.vector.tensor_tensor(
                        xpT[:, j, :],
                        quads[j].rearrange("p a b -> p (a b)"),
                        bias_sb[:, bias_col0 + j:bias_col0 + j + 1]
                        .to_broadcast((P, T)),
                        mybir.AluOpType.add,
                    )

            project(kT, wk_sb, 2, kpT, "k")
            project(qT, wq_sb, 0, qpT, "q")
            nc.sync.dma_start(wc_sb[:], Wc_r.ap())
            nc.sync.dma_start(ve_sb[:], vext.ap())

            # ---- attention per (head-pair j, q-chunk c) ----
            # Cross-unit software pipeline: each unit's last LAG av-matmuls
            # and its normalize chain are emitted after the NEXT unit's first
            # exp, so the ScalarE stream never waits at unit boundaries.
            LAG = 3

            def attn_unit(j, c):
                q0 = c * QC
                s_ps = ps.tile([P, 2, QC], F32, tag="big", name=f"s{j}{c}")
                y_ps = ps.tile([P, 2, QC], F32, tag="big", name=f"y{j}{c}")
                es_tiles = {}

                def s_mm(kk):
                    for s in range(2):  # head slot: partitions s*64..s*64+64
                        p0 = s * 64
                        for nch in range(2):
                            nc.tensor.matmul(
                                s_ps[:, s, nch * 512:(nch + 1) * 512],
                                kpT[p0:p0 + HD, j, kk * P:(kk + 1) * P],
                                qpT[p0:p0 + HD, j,
                                    q0 + nch * 512:q0 + (nch + 1) * 512],
                                start=True,
                                stop=True,
                            )

                def av_mm(kk):
                    es = es_tiles.pop(kk)
                    for s in range(2):
                        h = 2 * j + s
                        for nch in range(2):
                            nc.tensor.matmul(
                                y_ps[0:HD + 1, s, nch * 512:(nch + 1) * 512],
                                ve_sb[:, h, kk, :],
                                es[:, s, nch * 512:(nch + 1) * 512],
                                start=(kk == 0),
                                stop=(kk == NKV - 1),
                            )

                def finish():
                    for kk in range(NKV - LAG, NKV):
                        av_mm(kk)
                    # normalize: yallT[.., q0:q0+QC] = y/colsum, both heads:
                    # 1/colsum straight off PSUM row 64, one partition-
                    # broadcast for the pair, then per-head multiplies.
                    colsum2 = npool.tile([1, 2, QC], F32, tag="colsum",
                                         name=f"cs{j}{c}")
                    nc.vector.tensor_copy(colsum2[:], y_ps[HD:HD + 1, :, :])
                    recip2 = npool.tile([1, 2, QC], F32, tag="recip",
                                        name=f"rc{j}{c}")
                    nc.vector.reciprocal_approx_fast(
                        out=recip2[:], in_=colsum2[:]
                    )
                    bcast2 = npool.tile([HD, 2, QC], F32, tag="bcast",
                                        name=f"bc{j}{c}")
                    nc.gpsimd.partition_broadcast(bcast2[:], recip2[:])
                    nc.vector.tensor_tensor(
                        yallT[0:HD, j, q0:q0 + QC],
                        y_ps[0:HD, 0, :], bcast2[:, 0, :],
                        mybir.AluOpType.mult,
                    )
                    yn = npool.tile([HD, QC], BF16, tag="yn",
                                    name=f"yn{j}{c}")
                    nc.vector.tensor_tensor(
                        yn[:], y_ps[0:HD, 1, :], bcast2[:, 1, :],
                        mybir.AluOpType.mult,
                    )
                    nc.sync.dma_start(
                        yallT[64:64 + HD, j, q0:q0 + QC], yn[:]
                    )

                def step(kk, finish_prev):
                    s_mm(kk)
                    es = espool.tile([P, 2, QC], BF16, tag="es",
                                     name=f"e{j}{c}{kk}")
                    nc.scalar.activation(
                        es[:], s_ps[:], mybir.ActivationFunctionType.Exp,
                        scale=SCALE,
                    )
                    es_tiles[kk] = es
                    if finish_prev is not None and kk == 0:
                        finish_prev()
                    if kk >= LAG:
                        av_mm(kk - LAG)

                return step, finish

            units = [(j, c) for c in range(T // QC) for j in range(2)]
            finish_prev = None
            for (j, c) in units:
                step, finish = attn_unit(j, c)
                for kk in range(NKV):
                    step(kk, finish_prev)
                finish_prev = finish
            finish_prev()

            # ---- c_proj: out[t,:] = sum_j yallT[:,j,t].T @ wc[j] ----
            for tg in range(8):  # two t-tiles per psum alloc
                cp = ps.tile([P, 2, 2, 512], F32, tag="big", name=f"cp{tg}")
                for t2 in range(2):
                    tt = tg * 2 + t2
                    for j in range(2):
                        for nch in range(2):
                            nc.tensor.matmul(
                                cp[:, t2, nch, :],
                                yallT[:, j, tt * P:(tt + 1) * P],
                                wc_sb[:, j, nch * 512:(nch + 1) * 512],
                                start=(j == 0),
                                stop=(j == 1),
                            )
                for t2 in range(2):
                    tt = tg * 2 + t2
                    o_sb = opool.tile([P, D], BF16, tag="osb", name=f"o{tt}")
                    src = cp[:, t2, :, :].rearrange("p a b -> p (a b)")
                    if t2 == 0:
                        nc.scalar.copy(o_sb[:], src)
                    else:
                        nc.vector.tensor_copy(o_sb[:], src)
                    nc.sync.dma_start(out.ap()[tt * P:(tt + 1) * P, :], o_sb[:])

    nc.compile()
    _cache["nc"] = nc
    return nc


def make_in_maps(k, q, v, Wk, bk, Wq, bq, Wc, bc):
    bf = ml_dtypes.bfloat16
    k = np.asarray(k, dtype=np.float32)
    q = np.asarray(q, dtype=np.float32)
    v = np.asarray(v, dtype=np.float32)
    Wk = np.asarray(Wk, dtype=np.float32)
    Wq = np.asarray(Wq, dtype=np.float32)
    Wc = np.asarray(Wc, dtype=np.float32)
    bk = np.asarray(bk, dtype=np.float32)
    bq = np.asarray(bq, dtype=np.float32)
    in_maps = []
    for cidx in range(N_CORES):
        b = cidx // 4
        h0 = (cidx % 4) * HL
        sl = slice(h0 * HD, h0 * HD + DH)
        bq_t = np.ascontiguousarray(bq[sl].reshape(2, P).T)  # [128, 2]
        bk_t = np.ascontiguousarray(bk[sl].reshape(2, P).T)
        bqk = np.concatenate([bq_t, bk_t], axis=1)           # [128, 4]
        # vext [P, HL, NKV, HD+1]: [p, h, m, d] = v[m*128+p, sl][h*64+d],
        # ones at d=64 (colsum row for the av matmul)
        vsl = v[b][:, sl]                                    # [T, 256]
        ve = np.ones((P, HL, NKV, HD + 1), dtype=np.float32)
        ve[:, :, :, 0:HD] = (
            vsl.reshape(NKV, P, HL, HD).transpose(1, 2, 0, 3)
        )
        # Wq_r [128, 8, 256]: [p, i, m] = Wq[sl,:].T[i*128+p, m]
        wq_t = Wq[sl, :].T.reshape(8, P, DH).transpose(1, 0, 2)
        wk_t = Wk[sl, :].T.reshape(8, P, DH).transpose(1, 0, 2)
        wc_t = Wc[:, sl].T.reshape(2, P, D).transpose(1, 0, 2)
        in_maps.append({
            "qT": np.ascontiguousarray(q[b].T).astype(bf),
            "kT": np.ascontiguousarray(k[b].T).astype(bf),
            "vext": np.ascontiguousarray(ve).astype(bf),
            "Wq_r": np.ascontiguousarray(wq_t).astype(bf),
            "Wk_r": np.ascontiguousarray(wk_t).astype(bf),
            "Wc_r": np.ascontiguousarray(wc_t).astype(bf),
            "bqk": np.ascontiguousarray(bqk),
        })
    return in_maps


def kernel(k, q, v, Wk, bk, Wq, bq, Wc, bc, _trace=False, _trace_cores=None):
    bc = np.asarray(bc, dtype=np.float32)
    nc = build_nc()
    in_maps = make_in_maps(k, q, v, Wk, bk, Wq, bq, Wc, bc)
    res = run_bass_kernel_spmd(
        nc, in_maps, core_ids=list(range(N_CORES)),
        trace=_trace, trace_cores=_trace_cores,
    )
    outs = [res.results[c]["out"].astype(np.float32) for c in range(N_CORES)]
    full = np.stack([
        outs[0] + outs[1] + outs[2] + outs[3],
        outs[4] + outs[5] + outs[6] + outs[7],
    ]) + bc[None, None, :]
    kernel.last_result = res
    return full.astype(np.float32)


# revision 9
# speedup vs baseline: 1.6590x; 1.2079x over previous
"""Trainium2 Bass kernel for nn_CrossAttention (B=2, T=2048, D=1024, H=16, hd=64).

Sharding: 32 (batch, head) units over 8 cores -> each core handles 1 batch and
4 contiguous heads (core c: batch c//4, heads (c%4)*4 .. +4), grouped as two
head-pairs j in {0,1}.  Host sums the 4 partial c_proj outputs per batch and
adds bc.

Per-core dataflow (bf16 operands, D-on-partitions activations):
  qpT/kpT [128, 2, 2048] bf16 = W.T @ xT (+b), K=1024 accumulated in PSUM
  attention per (pair j, q-chunk 1024), pipelined at (kv-tile, q-half) grain:
    S half tile [128(kv), 2(head), 512(q)] f32 PSUM, double-buffered — the two
    heads' K=64 score matmuls run concurrently on PE row-groups 0/64, and the
    next half's matmuls overlap the current half's exp (no ScalarE gaps)
    exp: ScalarE activation over free-size-1024 chunks -> es bf16 SBUF
    av: [v_h | ones].T @ es accumulates y/colsum in [65, 2, 1024] f32
    norm: colsum -> reciprocal_approx_fast -> gpsimd partition_broadcast ->
    DVE multiply -> yallT bf16 (DMA partition-shift for odd heads); each
    unit's tail avs + norm are deferred past the next unit's first exp so
    the ScalarE stream never waits at unit boundaries
  cproj: out[t,:] = yallT.T @ WcT, K=256 accumulated in PSUM, bf16 out
"""

import sys

sys.path.insert(0, "/opt/trn_rl_repo")

import numpy as np
import ml_dtypes

import concourse.bacc as bacc
import concourse.bass as bass
import concourse.mybir as mybir
import concourse.tile as tile
from concourse.bass_utils import run_bass_kernel_spmd

F32 = mybir.dt.float32
BF16 = mybir.dt.bfloat16

T = 2048          # sequence length (q and kv)
D = 1024          # model dim
HL = 4            # heads per core
HD = 64           # head dim
DH = HL * HD      # 256 local projected dim
P = 128
QC = 1024         # q chunk for attention (y/norm granularity)
NKV = T // P      # 16 kv tiles
SCALE = 1.0 / 8.0  # 1/sqrt(64)
LAG = 4           # kv-steps by which av trails exp

N_CORES = 8

_cache = {}


def build_nc():
    if "nc" in _cache:
        return _cache["nc"]
    nc = bacc.Bacc(
        "TRN2",
        target_bir_lowering=False,
        debug=False,
        num_devices=N_CORES,
    )

    qT = nc.declare_dram_parameter("qT", [D, T], BF16, isOutput=False)
    kT = nc.declare_dram_parameter("kT", [D, T], BF16, isOutput=False)
    vext = nc.declare_dram_parameter(
        "vext", [P, HL, NKV, HD + 1], BF16, isOutput=False
    )
    Wq_r = nc.declare_dram_parameter("Wq_r", [P, 8, DH], BF16, isOutput=False)
    Wk_r = nc.declare_dram_parameter("Wk_r", [P, 8, DH], BF16, isOutput=False)
    Wc_r = nc.declare_dram_parameter("Wc_r", [P, 2, D], BF16, isOutput=False)
    bqk = nc.declare_dram_parameter("bqk", [P, 4], F32, isOutput=False)
    out = nc.declare_dram_parameter("out", [T, D], BF16, isOutput=True)

    with tile.TileContext(nc) as tc:
        with (
            tc.tile_pool(name="wpool", bufs=1) as wpool,
            tc.tile_pool(name="xpool", bufs=16) as xpool,
            tc.tile_pool(name="projsb", bufs=1) as projsb,
            tc.tile_pool(name="espool", bufs=10) as espool,
            tc.tile_pool(name="npool", bufs=1) as npool,
            tc.tile_pool(name="opool", bufs=3) as opool,
            tc.tile_pool(name="ps", bufs=1, space="PSUM") as ps,
        ):
            # ---- weights / constants (host pre-packed, contiguous DMA).
            bias_sb = wpool.tile([P, 4], F32, name="bias_sb")  # [bq0,bq1,bk0,bk1]
            nc.sync.dma_start(bias_sb[:], bqk.ap())
            wk_sb = wpool.tile([P, 8, DH], BF16, name="wk_sb")
            nc.sync.dma_start(wk_sb[:], Wk_r.ap())
            wq_sb = wpool.tile([P, 8, DH], BF16, name="wq_sb")
            nc.sync.dma_start(wq_sb[:], Wq_r.ap())
            wc_sb = wpool.tile([P, 2, D], BF16, name="wc_sb")
            ve_sb = wpool.tile([P, HL, NKV, HD + 1], BF16, name="ve_sb")

            # preload the exp activation table during the DMA ramp
            warm_sb = wpool.tile([P, 4], F32, name="warm_sb")
            nc.scalar.activation(
                warm_sb[:], bias_sb[:], mybir.ActivationFunctionType.Exp,
                scale=0.0,
            )

            kpT = projsb.tile([P, 2, T], BF16, name="kpT")
            qpT = projsb.tile([P, 2, T], BF16, name="qpT")
            yallT = projsb.tile([P, 2, T], BF16, name="yallT")

            # ---- input streams: issue all 16 tile DMAs up front ----
            xts = {}
            for nm, xd in (("k", kT), ("q", qT)):
                for i in range(8):
                    xt = xpool.tile([P, T], BF16, tag="xt", name=f"x{nm}{i}")
                    nc.sync.dma_start(xt[:], xd.ap()[i * P:(i + 1) * P, :])
                    xts[nm, i] = xt

            # ---- projections: xpT[j*128+p, t] = sum_i W[i,j].T @ x[i] ----
            # Two rotating [128, 2, 512] psum groups (tag "s", 2 banks each).
            def project(nm, w_sb, bias_col0, xpT):
                for j in range(2):
                    for tcp in range(2):
                        g = ps.tile([P, 2, 512], F32, tag="s", bufs=2,
                                    name=f"pg{nm}{j}{tcp}")
                        for i in range(8):
                            for t2 in range(2):
                                nc.tensor.matmul(
                                    g[:, t2, :],
                                    w_sb[:, i, j * P:(j + 1) * P],
                                    xts[nm, i][:,
                                               tcp * 1024 + t2 * 512:
                                               tcp * 1024 + (t2 + 1) * 512],
                                    start=(i == 0),
                                    stop=(i == 7),
                                )
                        nc.vector.tensor_tensor(
                            xpT[:, j, tcp * 1024:(tcp + 1) * 1024],
                            g.rearrange("p a b -> p (a b)"),
                            bias_sb[:, bias_col0 + j:bias_col0 + j + 1]
                            .to_broadcast((P, 1024)),
                            mybir.AluOpType.add,
                        )

            project("k", wk_sb, 2, kpT)
            project("q", wq_sb, 0, qpT)
            nc.sync.dma_start(wc_sb[:], Wc_r.ap())
            nc.sync.dma_start(ve_sb[:], vext.ap())

            # ---- attention per (head-pair j, q-chunk c) ----
            def attn_unit(j, c):
                q0 = c * QC
                y_ps = ps.tile([P, 2, QC], F32, tag="y", name=f"y{j}{c}")
                es_tiles = {}

                def step_half(kk, half):
                    qh = q0 + half * 512
                    s_ps = ps.tile([P, 2, 512], F32, tag="s", bufs=2,
                                   name=f"s{j}{c}{kk}{half}")
                    for s in range(2):  # head slot: partitions s*64..s*64+64
                        p0 = s * 64
                        nc.tensor.matmul(
                            s_ps[:, s, :],
                            kpT[p0:p0 + HD, j, kk * P:(kk + 1) * P],
                            qpT[p0:p0 + HD, j, qh:qh + 512],
                            start=True,
                            stop=True,
                        )
                    es = espool.tile([P, 2, 512], BF16, tag="es",
                                     name=f"e{j}{c}{kk}{half}")
                    nc.scalar.activation(
                        es[:], s_ps[:], mybir.ActivationFunctionType.Exp,
                        scale=SCALE,
                    )
                    es_tiles[kk, half] = es

                def av_mm(kk):
                    for s in range(2):
                        h = 2 * j + s
                        for half in range(2):
                            nc.tensor.matmul(
                                y_ps[0:HD + 1, s,
                                     half * 512:(half + 1) * 512],
                                ve_sb[:, h, kk, :],
                                es_tiles[kk, half][:, s, :],
                                start=(kk == 0),
                                stop=(kk == NKV - 1),
                            )
                    del es_tiles[kk, 0], es_tiles[kk, 1]

                def finish():
                    for kk in range(NKV - LAG, NKV):
                        av_mm(kk)
                    # normalize: yallT[.., q0:q0+QC] = y/colsum, both heads
                    colsum2 = npool.tile([1, 2, QC], F32, tag="colsum",
                                         name=f"cs{j}{c}")
                    nc.vector.tensor_copy(colsum2[:], y_ps[HD:HD + 1, :, :])
                    recip2 = npool.tile([1, 2, QC], F32, tag="recip",
                                        name=f"rc{j}{c}")
                    nc.vector.reciprocal_approx_fast(
                        out=recip2[:], in_=colsum2[:]
                    )
                    bcast2 = npool.tile([HD, 2, QC], F32, tag="bcast",
                                        name=f"bc{j}{c}")
                    nc.gpsimd.partition_broadcast(bcast2[:], recip2[:])
                    nc.vector.tensor_tensor(
                        yallT[0:HD, j, q0:q0 + QC],
                        y_ps[0:HD, 0, :], bcast2[:, 0, :],
                        mybir.AluOpType.mult,
                    )
                    yn = npool.tile([HD, QC], BF16, tag="yn", name=f"yn{j}{c}")
                    nc.vector.tensor_tensor(
                        yn[:], y_ps[0:HD, 1, :], bcast2[:, 1, :],
                        mybir.AluOpType.mult,
                    )
                    nc.sync.dma_start(
                        yallT[64:64 + HD, j, q0:q0 + QC], yn[:]
                    )

                def step(kk, finish_prev):
                    step_half(kk, 0)
                    step_half(kk, 1)
                    if finish_prev is not None and kk == 0:
                        finish_prev()
                    if kk >= LAG:
                        av_mm(kk - LAG)

                return step, finish

            units = [(j, c) for c in range(T // QC) for j in range(2)]
            finish_prev = None
            for (j, c) in units:
                step, finish = attn_unit(j, c)
                for kk in range(NKV):
                    step(kk, finish_prev)
                finish_prev = finish
            finish_prev()

            # ---- c_proj: out[t,:] = sum_j yallT[:,j,t].T @ wc[j] ----
            for tt in range(16):
                cp = ps.tile([P, 2, 512], F32, tag="s", bufs=2, name=f"cp{tt}")
                for j in range(2):
                    for nch in range(2):
                        nc.tensor.matmul(
                            cp[:, nch, :],
                            yallT[:, j, tt * P:(tt + 1) * P],
                            wc_sb[:, j, nch * 512:(nch + 1) * 512],
                            start=(j == 0),
                            stop=(j == 1),
                        )
                o_sb = opool.tile([P, D], BF16, tag="osb", name=f"o{tt}")
                src = cp.rearrange("p a b -> p (a b)")
                if tt % 2 == 0:
                    nc.scalar.copy(o_sb[:], src)
                else:
                    nc.vector.tensor_copy(o_sb[:], src)
                nc.sync.dma_start(out.ap()[tt * P:(tt + 1) * P, :], o_sb[:])

    nc.compile()
    _cache["nc"] = nc
    return nc


def make_in_maps(k, q, v, Wk, bk, Wq, bq, Wc, bc):
    bf = ml_dtypes.bfloat16
    k = np.asarray(k, dtype=np.float32)
    q = np.asarray(q, dtype=np.float32)
    v = np.asarray(v, dtype=np.float32)
    Wk = np.asarray(Wk, dtype=np.float32)
    Wq = np.asarray(Wq, dtype=np.float32)
    Wc = np.asarray(Wc, dtype=np.float32)
    bk = np.asarray(bk, dtype=np.float32)
    bq = np.asarray(bq, dtype=np.float32)
    in_maps = []
    for cidx in range(N_CORES):
        b = cidx // 4
        h0 = (cidx % 4) * HL
        sl = slice(h0 * HD, h0 * HD + DH)
        bq_t = np.ascontiguousarray(bq[sl].reshape(2, P).T)  # [128, 2]
        bk_t = np.ascontiguousarray(bk[sl].reshape(2, P).T)
        bqk = np.concatenate([bq_t, bk_t], axis=1)           # [128, 4]
        # vext [P, HL, NKV, HD+1]: [p, h, m, d] = v[m*128+p, sl][h*64+d],
        # ones at d=64 (colsum row for the av matmul)
        vsl = v[b][:, sl]                                    # [T, 256]
        ve = np.ones((P, HL, NKV, HD + 1), dtype=np.float32)
        ve[:, :, :, 0:HD] = (
            vsl.reshape(NKV, P, HL, HD).transpose(1, 2, 0, 3)
        )
        # Wq_r [128, 8, 256]: [p, i, m] = Wq[sl,:].T[i*128+p, m]
        wq_t = Wq[sl, :].T.reshape(8, P, DH).transpose(1, 0, 2)
        wk_t = Wk[sl, :].T.reshape(8, P, DH).transpose(1, 0, 2)
        wc_t = Wc[:, sl].T.reshape(2, P, D).transpose(1, 0, 2)
        in_maps.append({
            "qT": np.ascontiguousarray(q[b].T).astype(bf),
            "kT": np.ascontiguousarray(k[b].T).astype(bf),
            "vext": np.ascontiguousarray(ve).astype(bf),
            "Wq_r": np.ascontiguousarray(wq_t).astype(bf),
            "Wk_r": np.ascontiguousarray(wk_t).astype(bf),
            "Wc_r": np.ascontiguousarray(wc_t).astype(bf),
            "bqk": np.ascontiguousarray(bqk),
        })
    return in_maps


def kernel(k, q, v, Wk, bk, Wq, bq, Wc, bc, _trace=False, _trace_cores=None):
    bc = np.asarray(bc, dtype=np.float32)
    nc = build_nc()
    in_maps = make_in_maps(k, q, v, Wk, bk, Wq, bq, Wc, bc)
    res = run_bass_kernel_spmd(
        nc, in_maps, core_ids=list(range(N_CORES)),
        trace=_trace, trace_cores=_trace_cores,
    )
    outs = [res.results[c]["out"].astype(np.float32) for c in range(N_CORES)]
    full = np.stack([
        outs[0] + outs[1] + outs[2] + outs[3],
        outs[4] + outs[5] + outs[6] + outs[7],
    ]) + bc[None, None, :]
    kernel.last_result = res
    return full.astype(np.float32)


# revision 12
# speedup vs baseline: 1.7258x; 1.0403x over previous
"""Trainium2 Bass kernel for nn_CrossAttention (B=2, T=2048, D=1024, H=16, hd=64).

Sharding: 32 (batch, head) units over 8 cores -> each core handles 1 batch and
4 contiguous heads (core c: batch c//4, heads (c%4)*4 .. +4), grouped as two
head-pairs j in {0,1}.  Host sums the 4 partial c_proj outputs per batch and
adds bc.

Per-core dataflow (bf16 operands, D-on-partitions activations), organized as
one long ScalarE exp stream that everything else hides behind:
  qpT/kpT [128, 2, 2048] bf16 = W.T @ xT (+b), K=1024 accumulated in PSUM.
  Attention runs per (pair j, q-chunk 512) at kv-tile grain: the two heads'
  K=64 score matmuls go to PE row-groups 0/64 concurrently into a double-
  buffered [128, 2, 512] f32 S tile; one exp (free-size 1024) -> es bf16;
  av matmuls ([v_h | ones].T @ es) accumulate y+colsum into [65, 2, 512]
  f32, lagging exp by LAG kv-steps.  Each unit's trailing avs are spread
  over the next unit's first steps; y is evacuated to SBUF early so the
  normalize chain (reciprocal_approx_fast -> gpsimd partition_broadcast ->
  multiply) never blocks PSUM reuse.  PSUM: S 2x2 banks + y 2 banks + a
  2-bank "aux" slot on which projection groups and c_proj t-tiles run as
  small "filler" bundles inside the attention steps, so only 3 projection
  groups precede the exp stream and only the last c_proj tiles follow it.
"""

import sys

sys.path.insert(0, "/opt/trn_rl_repo")

from collections import deque

import numpy as np
import ml_dtypes

import concourse.bacc as bacc
import concourse.bass as bass
import concourse.mybir as mybir
import concourse.tile as tile
from concourse.bass_utils import run_bass_kernel_spmd

F32 = mybir.dt.float32
BF16 = mybir.dt.bfloat16

T = 2048          # sequence length (q and kv)
D = 1024          # model dim
HL = 4            # heads per core
HD = 64           # head dim
DH = HL * HD      # 256 local projected dim
P = 128
QC = 512          # q chunk for attention (y/norm granularity)
NU = T // QC      # 4 q-chunks
NKV = T // P      # 16 kv tiles
SCALE = 1.0 / 8.0  # 1/sqrt(64)
LAG = 8           # kv-steps by which av trails exp

N_CORES = 8

_cache = {}


def build_nc():
    if "nc" in _cache:
        return _cache["nc"]
    nc = bacc.Bacc(
        "TRN2",
        target_bir_lowering=False,
        debug=False,
        num_devices=N_CORES,
    )

    qT = nc.declare_dram_parameter("qT", [D, T], BF16, isOutput=False)
    kT = nc.declare_dram_parameter("kT", [D, T], BF16, isOutput=False)
    vext = nc.declare_dram_parameter(
        "vext", [P, HL, NKV, HD + 1], BF16, isOutput=False
    )
    Wq_r = nc.declare_dram_parameter("Wq_r", [P, 8, DH], BF16, isOutput=False)
    Wk_r = nc.declare_dram_parameter("Wk_r", [P, 8, DH], BF16, isOutput=False)
    Wc_r = nc.declare_dram_parameter("Wc_r", [P, 2, D], BF16, isOutput=False)
    bqk = nc.declare_dram_parameter("bqk", [P, 4], F32, isOutput=False)
    out = nc.declare_dram_parameter("out", [T, D], BF16, isOutput=True)

    with tile.TileContext(nc) as tc:
        with (
            tc.tile_pool(name="wpool", bufs=1) as wpool,
            tc.tile_pool(name="xpool", bufs=16) as xpool,
            tc.tile_pool(name="projsb", bufs=1) as projsb,
            tc.tile_pool(name="espool", bufs=12) as espool,
            tc.tile_pool(name="npool", bufs=2) as npool,
            tc.tile_pool(name="opool", bufs=3) as opool,
            tc.tile_pool(name="ps", bufs=1, space="PSUM") as ps,
        ):
            # ---- weights / constants (host pre-packed, contiguous DMA) ----
            bias_sb = wpool.tile([P, 4], F32, name="bias_sb")  # [bq0,bq1,bk0,bk1]
            nc.sync.dma_start(bias_sb[:], bqk.ap())
            wk_sb = wpool.tile([P, 8, DH], BF16, name="wk_sb")
            nc.sync.dma_start(wk_sb[:], Wk_r.ap())
            wq_sb = wpool.tile([P, 8, DH], BF16, name="wq_sb")
            nc.sync.dma_start(wq_sb[:], Wq_r.ap())
            wc_sb = wpool.tile([P, 2, D], BF16, name="wc_sb")
            ve_sb = wpool.tile([P, HL, NKV, HD + 1], BF16, name="ve_sb")

            # preload the exp activation table during the DMA ramp
            warm_sb = wpool.tile([P, 4], F32, name="warm_sb")
            nc.scalar.activation(
                warm_sb[:], bias_sb[:], mybir.ActivationFunctionType.Exp,
                scale=0.0,
            )

            kpT = projsb.tile([P, 2, T], BF16, name="kpT")
            qpT = projsb.tile([P, 2, T], BF16, name="qpT")
            yallT = projsb.tile([P, 2, T], BF16, name="yallT")

            # ---- input streams: issue all 16 tile DMAs up front ----
            xts = {}
            for nm, xd in (("k", kT), ("q", qT)):
                for i in range(8):
                    xt = xpool.tile([P, T], BF16, tag="xt", name=f"x{nm}{i}")
                    nc.sync.dma_start(xt[:], xd.ap()[i * P:(i + 1) * P, :])
                    xts[nm, i] = xt
            nc.sync.dma_start(ve_sb[:], vext.ap())
            nc.sync.dma_start(wc_sb[:], Wc_r.ap())

            projw = {"k": (wk_sb, 2, kpT), "q": (wq_sb, 0, qpT)}

            def pgroup_closures(nm, j, tcp):
                """One projection PSUM group = 16 accumulating matmuls +
                bias evac on the aux slot, chopped into <=3-matmul filler
                closures.  Lifecycles on aux are strictly sequential."""
                w_sb, bias_col0, xpT = projw[nm]
                st = {}
                mms = [(i, t2) for i in range(8) for t2 in range(2)]

                def emit(lo, hi):
                    def go():
                        if "g" not in st:
                            st["g"] = ps.tile([P, 2, 512], F32, tag="aux",
                                              name=f"pg{nm}{j}{tcp}")
                        for i, t2 in mms[lo:hi]:
                            nc.tensor.matmul(
                                st["g"][:, t2, :],
                                w_sb[:, i, j * P:(j + 1) * P],
                                xts[nm, i][:,
                                           tcp * 1024 + t2 * 512:
                                           tcp * 1024 + (t2 + 1) * 512],
                                start=(i == 0),
                                stop=(i == 7),
                            )
                    return go

                def evac():
                    nc.vector.tensor_tensor(
                        xpT[:, j, tcp * 1024:(tcp + 1) * 1024],
                        st["g"].rearrange("p a b -> p (a b)"),
                        bias_sb[:, bias_col0 + j:bias_col0 + j + 1]
                        .to_broadcast((P, 1024)),
                        mybir.AluOpType.add,
                    )

                cs = [emit(lo, min(lo + 3, 16)) for lo in range(0, 16, 3)]
                cs.append(evac)
                return cs

            def cp_closure(tt):
                """One c_proj t-tile on the aux slot: 4 matmuls (K=256 over
                j) + evac + DMA, as a single small filler closure."""
                def go():
                    cp = ps.tile([P, 2, 512], F32, tag="aux", name=f"cp{tt}")
                    for j in range(2):
                        for nch in range(2):
                            nc.tensor.matmul(
                                cp[:, nch, :],
                                yallT[:, j, tt * P:(tt + 1) * P],
                                wc_sb[:, j, nch * 512:(nch + 1) * 512],
                                start=(j == 0),
                                stop=(j == 1),
                            )
                    o_sb = opool.tile([P, D], BF16, tag="osb", name=f"o{tt}")
                    nc.vector.tensor_copy(
                        o_sb[:], cp.rearrange("p a b -> p (a b)")
                    )
                    nc.sync.dma_start(
                        out.ap()[tt * P:(tt + 1) * P, :], o_sb[:]
                    )
                return go

            # inline projection prologue: what attention units (0,0)/(1,0)
            # need first (q j0 tcp0 covers q-chunks 0 and 1)
            for cl in (pgroup_closures("k", 0, 0) + pgroup_closures("k", 0, 1)
                       + pgroup_closures("q", 0, 0)):
                cl()

            filler = deque()
            for nm, j, tcp in (("k", 1, 0), ("q", 1, 0), ("k", 1, 1),
                               ("q", 0, 1), ("q", 1, 1)):
                filler.extend(pgroup_closures(nm, j, tcp))

            # ---- attention units ----
            def attn_unit(j, c):
                q0 = c * QC
                st = {}
                es_tiles = {}

                def step_mm(kk):
                    s_ps = ps.tile([P, 2, 512], F32, tag="s", bufs=2,
                                   name=f"s{j}{c}{kk}")
                    for s in range(2):  # head slot: partitions s*64..s*64+64
                        p0 = s * 64
                        nc.tensor.matmul(
                            s_ps[:, s, :],
                            kpT[p0:p0 + HD, j, kk * P:(kk + 1) * P],
                            qpT[p0:p0 + HD, j, q0:q0 + QC],
                            start=True,
                            stop=True,
                        )
                    es = espool.tile([P, 2, QC], BF16, tag="es",
                                     name=f"e{j}{c}{kk}")
                    nc.scalar.activation(
                        es[:], s_ps[:], mybir.ActivationFunctionType.Exp,
                        scale=SCALE,
                    )
                    es_tiles[kk] = es

                def av_mm(kk):
                    if "y" not in st:
                        st["y"] = ps.tile([P, 2, QC], F32, tag="y",
                                          name=f"y{j}{c}")
                    for s in range(2):
                        h = 2 * j + s
                        nc.tensor.matmul(
                            st["y"][0:HD + 1, s, :],
                            ve_sb[:, h, kk, :],
                            es_tiles[kk][:, s, :],
                            start=(kk == 0),
                            stop=(kk == NKV - 1),
                        )
                    del es_tiles[kk]

                def norm():
                    # evacuate y+colsum to SBUF first (frees the PSUM slot),
                    # then normalize off-PSUM.
                    y_ps = st["y"]
                    colsum2 = npool.tile([1, 2, QC], F32, tag="colsum",
                                         name=f"cs{j}{c}")
                    nc.vector.tensor_copy(colsum2[:], y_ps[HD:HD + 1, :, :])
                    yev = npool.tile([HD, 2, QC], F32, tag="yev",
                                     name=f"ye{j}{c}")
                    nc.vector.tensor_copy(yev[:], y_ps[0:HD, :, :])
                    recip2 = npool.tile([1, 2, QC], F32, tag="recip",
                                        name=f"rc{j}{c}")
                    nc.vector.reciprocal_approx_fast(
                        out=recip2[:], in_=colsum2[:]
                    )
                    bcast2 = npool.tile([HD, 2, QC], F32, tag="bcast",
                                        name=f"bc{j}{c}")
                    nc.gpsimd.partition_broadcast(bcast2[:], recip2[:])
                    nc.vector.tensor_tensor(
                        yallT[0:HD, j, q0:q0 + QC],
                        yev[:, 0, :], bcast2[:, 0, :],
                        mybir.AluOpType.mult,
                    )
                    yn = npool.tile([HD, QC], BF16, tag="yn", name=f"yn{j}{c}")
                    nc.vector.tensor_tensor(
                        yn[:], yev[:, 1, :], bcast2[:, 1, :],
                        mybir.AluOpType.mult,
                    )
                    nc.sync.dma_start(
                        yallT[64:64 + HD, j, q0:q0 + QC], yn[:]
                    )

                return step_mm, av_mm, norm

            units = [(j, c) for c in range(NU) for j in range(2)]
            prev = None
            for j, c in units:
                step_mm, av_mm, norm = attn_unit(j, c)
                for kk in range(NKV):
                    step_mm(kk)
                    if prev is not None:
                        if kk < 4:
                            prev["av"](2 * kk)
                            prev["av"](2 * kk + 1)
                        elif kk == 4:
                            prev["norm"]()
                            if j == 0 and c >= 1:
                                # q-chunk c-1 rows of yallT complete
                                for tt in range(4 * (c - 1), 4 * c):
                                    filler.append(cp_closure(tt))
                    if kk >= LAG:
                        av_mm(kk - LAG)
                    # pops start at kk=1 so every filler group's last closure
                    # is emitted strictly before its first consumer step
                    if kk >= 1 and filler:
                        filler.popleft()()
                prev = {
                    "av": lambda i, f=av_mm: f(NKV - LAG + i),
                    "norm": norm,
                }
            for i in range(LAG):
                prev["av"](i)
            prev["norm"]()
            for tt in range(4 * (NU - 1), 4 * NU):
                filler.append(cp_closure(tt))
            while filler:
                filler.popleft()()

    nc.compile()
    _cache["nc"] = nc
    return nc


def make_in_maps(k, q, v, Wk, bk, Wq, bq, Wc, bc):
    bf = ml_dtypes.bfloat16
    k = np.asarray(k, dtype=np.float32)
    q = np.asarray(q, dtype=np.float32)
    v = np.asarray(v, dtype=np.float32)
    Wk = np.asarray(Wk, dtype=np.float32)
    Wq = np.asarray(Wq, dtype=np.float32)
    Wc = np.asarray(Wc, dtype=np.float32)
    bk = np.asarray(bk, dtype=np.float32)
    bq = np.asarray(bq, dtype=np.float32)
    in_maps = []
    for cidx in range(N_CORES):
        b = cidx // 4
        h0 = (cidx % 4) * HL
        sl = slice(h0 * HD, h0 * HD + DH)
        bq_t = np.ascontiguousarray(bq[sl].reshape(2, P).T)  # [128, 2]
        bk_t = np.ascontiguousarray(bk[sl].reshape(2, P).T)
        bqk = np.concatenate([bq_t, bk_t], axis=1)           # [128, 4]
        # vext [P, HL, NKV, HD+1]: [p, h, m, d] = v[m*128+p, sl][h*64+d],
        # ones at d=64 (colsum row for the av matmul)
        vsl = v[b][:, sl]                                    # [T, 256]
        ve = np.ones((P, HL, NKV, HD + 1), dtype=np.float32)
        ve[:, :, :, 0:HD] = (
            vsl.reshape(NKV, P, HL, HD).transpose(1, 2, 0, 3)
        )
        # Wq_r [128, 8, 256]: [p, i, m] = Wq[sl,:].T[i*128+p, m]
        wq_t = Wq[sl, :].T.reshape(8, P, DH).transpose(1, 0, 2)
        wk_t = Wk[sl, :].T.reshape(8, P, DH).transpose(1, 0, 2)
        wc_t = Wc[:, sl].T.reshape(2, P, D).transpose(1, 0, 2)
        in_maps.append({
            "qT": np.ascontiguousarray(q[b].T).astype(bf),
            "kT": np.ascontiguousarray(k[b].T).astype(bf),
            "vext": np.ascontiguousarray(ve).astype(bf),
            "Wq_r": np.ascontiguousarray(wq_t).astype(bf),
            "Wk_r": np.ascontiguousarray(wk_t).astype(bf),
            "Wc_r": np.ascontiguousarray(wc_t).astype(bf),
            "bqk": np.ascontiguousarray(bqk),
        })
    return in_maps


def kernel(k, q, v, Wk, bk, Wq, bq, Wc, bc, _trace=False, _trace_cores=None):
    bc = np.asarray(bc, dtype=np.float32)
    nc = build_nc()
    in_maps = make_in_maps(k, q, v, Wk, bk, Wq, bq, Wc, bc)
    res = run_bass_kernel_spmd(
        nc, in_maps, core_ids=list(range(N_CORES)),
        trace=_trace, trace_cores=_trace_cores,
    )
    outs = [res.results[c]["out"].astype(np.float32) for c in range(N_CORES)]
    full = np.stack([
        outs[0] + outs[1] + outs[2] + outs[3],
        outs[4] + outs[5] + outs[6] + outs[7],
    ]) + bc[None, None, :]
    kernel.last_result = res
    return full.astype(np.float32)


# revision 16
# speedup vs baseline: 1.7424x; 1.0096x over previous
"""Trainium2 Bass kernel for nn_CrossAttention (B=2, T=2048, D=1024, H=16, hd=64).

Sharding: 32 (batch, head) units over 8 cores -> each core handles 1 batch and
4 contiguous heads (core c: batch c//4, heads (c%4)*4 .. +4), grouped as two
head-pairs j in {0,1}.  Host sums the 4 partial c_proj outputs per batch and
adds bc.

Per-core dataflow (bf16 operands, D-on-partitions activations), organized as
one long ScalarE exp stream that everything else hides behind:
  qpT/kpT [128, 2, 2048] bf16 = W.T @ xT (+b), K=1024 accumulated in PSUM.
  Attention runs per (pair j, q-chunk 512) at kv-tile grain: the two heads'
  K=64 score matmuls go to PE row-groups 0/64 concurrently into a double-
  buffered [128, 2, 512] f32 S tile; one exp (free-size 1024) -> es bf16;
  av matmuls ([v_h | ones].T @ es) accumulate y+colsum into [65, 2, 512]
  f32, lagging exp by LAG kv-steps.  Each unit's trailing avs are spread
  over the next unit's first steps; y is evacuated to SBUF early so the
  normalize chain (reciprocal_approx_fast -> gpsimd partition_broadcast ->
  multiply) never blocks PSUM reuse.  PSUM: S 2x2 banks + y 2 banks + a
  2-bank "aux" slot on which projection groups and c_proj t-tiles run as
  small "filler" bundles inside the attention steps, so only 3 projection
  groups precede the exp stream and only the last c_proj tiles follow it.
"""

import sys

sys.path.insert(0, "/opt/trn_rl_repo")

from collections import deque

import numpy as np
import ml_dtypes

import concourse.bacc as bacc
import concourse.bass as bass
import concourse.mybir as mybir
import concourse.tile as tile
from concourse.bass_utils import run_bass_kernel_spmd

F32 = mybir.dt.float32
BF16 = mybir.dt.bfloat16

T = 2048          # sequence length (q and kv)
D = 1024          # model dim
HL = 4            # heads per core
HD = 64           # head dim
DH = HL * HD      # 256 local projected dim
P = 128
QC = 512          # q chunk for attention (y/norm granularity)
NU = T // QC      # 4 q-chunks
NKV = T // P      # 16 kv tiles
SCALE = 1.0 / 8.0  # 1/sqrt(64)
LAG = 10          # kv-steps by which av trails exp

N_CORES = 8

_cache = {}


def build_nc():
    if "nc" in _cache:
        return _cache["nc"]
    nc = bacc.Bacc(
        "TRN2",
        target_bir_lowering=False,
        debug=False,
        num_devices=N_CORES,
    )

    qT = nc.declare_dram_parameter("qT", [D, T], BF16, isOutput=False)
    kT = nc.declare_dram_parameter("kT", [D, T], BF16, isOutput=False)
    vext = nc.declare_dram_parameter(
        "vext", [P, HL, NKV, HD + 1], BF16, isOutput=False
    )
    Wq_r = nc.declare_dram_parameter("Wq_r", [P, 8, DH], BF16, isOutput=False)
    Wk_r = nc.declare_dram_parameter("Wk_r", [P, 8, DH], BF16, isOutput=False)
    Wc_r = nc.declare_dram_parameter("Wc_r", [P, 2, D], BF16, isOutput=False)
    bqk = nc.declare_dram_parameter("bqk", [P, 4], F32, isOutput=False)
    out = nc.declare_dram_parameter("out", [T, D], BF16, isOutput=True)

    with tile.TileContext(nc) as tc:
        with (
            tc.tile_pool(name="wpool", bufs=1) as wpool,
            tc.tile_pool(name="xpool", bufs=16) as xpool,
            tc.tile_pool(name="projsb", bufs=1) as projsb,
            tc.tile_pool(name="espool", bufs=12) as espool,
            tc.tile_pool(name="npool", bufs=2) as npool,
            tc.tile_pool(name="opool", bufs=3) as opool,
            tc.tile_pool(name="ps", bufs=1, space="PSUM") as ps,
        ):
            # ---- weights / constants (host pre-packed, contiguous DMA) ----
            bias_sb = wpool.tile([P, 4], F32, name="bias_sb")  # [bq0,bq1,bk0,bk1]
            nc.sync.dma_start(bias_sb[:], bqk.ap())
            wk_sb = wpool.tile([P, 8, DH], BF16, name="wk_sb")
            nc.sync.dma_start(wk_sb[:], Wk_r.ap())
            wq_sb = wpool.tile([P, 8, DH], BF16, name="wq_sb")
            nc.sync.dma_start(wq_sb[:], Wq_r.ap())
            wc_sb = wpool.tile([P, 2, D], BF16, name="wc_sb")
            ve_sb = wpool.tile([P, HL, NKV, HD + 1], BF16, name="ve_sb")

            # preload the exp activation table during the DMA ramp
            warm_sb = wpool.tile([P, 4], F32, name="warm_sb")
            nc.scalar.activation(
                warm_sb[:], bias_sb[:], mybir.ActivationFunctionType.Exp,
                scale=0.0,
            )

            kpT = projsb.tile([P, 2, T], BF16, name="kpT")
            qpT = projsb.tile([P, 2, T], BF16, name="qpT")
            yallT = projsb.tile([P, 2, T], BF16, name="yallT")

            # ---- input streams: issue all 16 tile DMAs up front ----
            xts = {}
            for nm, xd in (("k", kT), ("q", qT)):
                for i in range(8):
                    xt = xpool.tile([P, T], BF16, tag="xt", name=f"x{nm}{i}")
                    nc.sync.dma_start(xt[:], xd.ap()[i * P:(i + 1) * P, :])
                    xts[nm, i] = xt
            nc.sync.dma_start(ve_sb[:], vext.ap())
            nc.sync.dma_start(wc_sb[:], Wc_r.ap())

            projw = {"k": (wk_sb, 2, kpT), "q": (wq_sb, 0, qpT)}

            def pgroup_closures(nm, j, tcp, tag="aux", bufs=1):
                """One projection PSUM group = 16 accumulating matmuls +
                bias evac on the aux slot, chopped into <=3-matmul filler
                closures.  Lifecycles on aux are strictly sequential."""
                w_sb, bias_col0, xpT = projw[nm]
                st = {}
                mms = [(i, t2) for i in range(8) for t2 in range(2)]

                def emit(lo, hi):
                    def go():
                        if "g" not in st:
                            st["g"] = ps.tile([P, 2, 512], F32, tag=tag,
                                              bufs=bufs,
                                              name=f"pg{nm}{j}{tcp}")
                        for i, t2 in mms[lo:hi]:
                            nc.tensor.matmul(
                                st["g"][:, t2, :],
                                w_sb[:, i, j * P:(j + 1) * P],
                                xts[nm, i][:,
                                           tcp * 1024 + t2 * 512:
                                           tcp * 1024 + (t2 + 1) * 512],
                                start=(i == 0),
                                stop=(i == 7),
                            )
                    return go

                def evac():
                    nc.vector.tensor_tensor(
                        xpT[:, j, tcp * 1024:(tcp + 1) * 1024],
                        st["g"].rearrange("p a b -> p (a b)"),
                        bias_sb[:, bias_col0 + j:bias_col0 + j + 1]
                        .to_broadcast((P, 1024)),
                        mybir.AluOpType.add,
                    )

                cs = [emit(lo, min(lo + 3, 16)) for lo in range(0, 16, 3)]
                cs.append(evac)
                return cs

            def cp_closure(tt):
                """One c_proj t-tile on the aux slot: 4 matmuls (K=256 over
                j) + evac + DMA, as a single small filler closure."""
                def go():
                    cp = ps.tile([P, 2, 512], F32, tag="aux", name=f"cp{tt}")
                    for j in range(2):
                        for nch in range(2):
                            nc.tensor.matmul(
                                cp[:, nch, :],
                                yallT[:, j, tt * P:(tt + 1) * P],
                                wc_sb[:, j, nch * 512:(nch + 1) * 512],
                                start=(j == 0),
                                stop=(j == 1),
                            )
                    o_sb = opool.tile([P, D], BF16, tag="osb", name=f"o{tt}")
                    nc.vector.tensor_copy(
                        o_sb[:], cp.rearrange("p a b -> p (a b)")
                    )
                    nc.sync.dma_start(
                        out.ap()[tt * P:(tt + 1) * P, :], o_sb[:]
                    )
                return go

            # inline projection prologue: what attention units (0,0)/(1,0)
            # need first (q j0 tcp0 covers q-chunks 0 and 1).  The k groups
            # run on the two "s" slots (still free) so they overlap.
            for cl in (pgroup_closures("k", 0, 0, tag="s", bufs=2)
                       + pgroup_closures("k", 0, 1, tag="s", bufs=2)
                       + pgroup_closures("q", 0, 0)):
                cl()

            filler = deque()
            for nm, j, tcp in (("k", 1, 0), ("q", 1, 0), ("k", 1, 1),
                               ("q", 0, 1), ("q", 1, 1)):
                filler.extend(pgroup_closures(nm, j, tcp))

            # ---- attention units ----
            def attn_unit(j, c):
                q0 = c * QC
                st = {}
                es_tiles = {}

                def step_mm(kk):
                    s_ps = ps.tile([P, 2, 512], F32, tag="s", bufs=2,
                                   name=f"s{j}{c}{kk}")
                    for s in range(2):  # head slot: partitions s*64..s*64+64
                        p0 = s * 64
                        nc.tensor.matmul(
                            s_ps[:, s, :],
                            kpT[p0:p0 + HD, j, kk * P:(kk + 1) * P],
                            qpT[p0:p0 + HD, j, q0:q0 + QC],
                            start=True,
                            stop=True,
                        )
                    es = espool.tile([P, 2, QC], BF16, tag="es",
                                     name=f"e{j}{c}{kk}")
                    nc.scalar.activation(
                        es[:], s_ps[:], mybir.ActivationFunctionType.Exp,
                        scale=SCALE,
                    )
                    es_tiles[kk] = es

                def av_mm(kk):
                    if "y" not in st:
                        st["y"] = ps.tile([P, 2, QC], F32, tag="y",
                                          name=f"y{j}{c}")
                    for s in range(2):
                        h = 2 * j + s
                        nc.tensor.matmul(
                            st["y"][0:HD + 1, s, :],
                            ve_sb[:, h, kk, :],
                            es_tiles[kk][:, s, :],
                            start=(kk == 0),
                            stop=(kk == NKV - 1),
                        )
                    del es_tiles[kk]

                def norm():
                    # evacuate y+colsum to SBUF first (frees the PSUM slot),
                    # then normalize off-PSUM.
                    y_ps = st["y"]
                    colsum2 = npool.tile([1, 2, QC], F32, tag="colsum",
                                         name=f"cs{j}{c}")
                    nc.vector.tensor_copy(colsum2[:], y_ps[HD:HD + 1, :, :])
                    yev = npool.tile([HD, 2, QC], F32, tag="yev",
                                     name=f"ye{j}{c}")
                    nc.vector.tensor_copy(yev[:], y_ps[0:HD, :, :])
                    recip2 = npool.tile([1, 2, QC], F32, tag="recip",
                                        name=f"rc{j}{c}")
                    nc.vector.reciprocal_approx_fast(
                        out=recip2[:], in_=colsum2[:]
                    )
                    bcast2 = npool.tile([HD, 2, QC], F32, tag="bcast",
                                        name=f"bc{j}{c}")
                    nc.gpsimd.partition_broadcast(bcast2[:], recip2[:])
                    nc.vector.tensor_tensor(
                        yallT[0:HD, j, q0:q0 + QC],
                        yev[:, 0, :], bcast2[:, 0, :],
                        mybir.AluOpType.mult,
                    )
                    yn = npool.tile([HD, QC], BF16, tag="yn", name=f"yn{j}{c}")
                    nc.vector.tensor_tensor(
                        yn[:], yev[:, 1, :], bcast2[:, 1, :],
                        mybir.AluOpType.mult,
                    )
                    nc.sync.dma_start(
                        yallT[64:64 + HD, j, q0:q0 + QC], yn[:]
                    )

                return step_mm, av_mm, norm

            units = [(j, c) for c in range(NU) for j in range(2)]
            prev = None
            for j, c in units:
                step_mm, av_mm, norm = attn_unit(j, c)
                for kk in range(NKV):
                    step_mm(kk)
                    if prev is not None:
                        # spread the previous unit's 10 trailing avs over
                        # kk 0..8 (2 on kk=0), then its normalize at kk=8 —
                        # its y-slot is free well before our av(0) at kk=LAG
                        if kk == 0:
                            prev["avs"].popleft()()
                        if kk <= 8 and prev["avs"]:
                            prev["avs"].popleft()()
                        if kk == 8:
                            prev["norm"]()
                            if j == 0 and c >= 1:
                                # q-chunk c-1 rows of yallT complete
                                for tt in range(4 * (c - 1), 4 * c):
                                    filler.append(cp_closure(tt))
                    if kk >= LAG:
                        av_mm(kk - LAG)
                    # pops start at kk=1 so every filler group's last closure
                    # is emitted strictly before its first consumer step
                    if kk >= 1 and filler:
                        filler.popleft()()
                prev = {
                    "avs": deque(
                        (lambda kk2=kk2, f=av_mm: f(kk2))
                        for kk2 in range(NKV - LAG, NKV)
                    ),
                    "norm": norm,
                }
            while prev["avs"]:
                prev["avs"].popleft()()
            prev["norm"]()
            for tt in range(4 * (NU - 1), 4 * NU):
                filler.append(cp_closure(tt))
            while filler:
                filler.popleft()()

    nc.compile()
    _cache["nc"] = nc
    return nc


def make_in_maps(k, q, v, Wk, bk, Wq, bq, Wc, bc):
    bf = ml_dtypes.bfloat16
    k = np.asarray(k, dtype=np.float32)
    q = np.asarray(q, dtype=np.float32)
    v = np.asarray(v, dtype=np.float32)
    Wk = np.asarray(Wk, dtype=np.float32)
    Wq = np.asarray(Wq, dtype=np.float32)
    Wc = np.asarray(Wc, dtype=np.float32)
    bk = np.asarray(bk, dtype=np.float32)
    bq = np.asarray(bq, dtype=np.float32)
    in_maps = []
    for cidx in range(N_CORES):
        b = cidx // 4
        h0 = (cidx % 4) * HL
        sl = slice(h0 * HD, h0 * HD + DH)
        bq_t = np.ascontiguousarray(bq[sl].reshape(2, P).T)  # [128, 2]
        bk_t = np.ascontiguousarray(bk[sl].reshape(2, P).T)
        bqk = np.concatenate([bq_t, bk_t], axis=1)           # [128, 4]
        # vext [P, HL, NKV, HD+1]: [p, h, m, d] = v[m*128+p, sl][h*64+d],
        # ones at d=64 (colsum row for the av matmul)
        vsl = v[b][:, sl]                                    # [T, 256]
        ve = np.ones((P, HL, NKV, HD + 1), dtype=np.float32)
        ve[:, :, :, 0:HD] = (
            vsl.reshape(NKV, P, HL, HD).transpose(1, 2, 0, 3)
        )
        # Wq_r [128, 8, 256]: [p, i, m] = Wq[sl,:].T[i*128+p, m]
        wq_t = Wq[sl, :].T.reshape(8, P, DH).transpose(1, 0, 2)
        wk_t = Wk[sl, :].T.reshape(8, P, DH).transpose(1, 0, 2)
        wc_t = Wc[:, sl].T.reshape(2, P, D).transpose(1, 0, 2)
        in_maps.append({
            "qT": np.ascontiguousarray(q[b].T).astype(bf),
            "kT": np.ascontiguousarray(k[b].T).astype(bf),
            "vext": np.ascontiguousarray(ve).astype(bf),
            "Wq_r": np.ascontiguousarray(wq_t).astype(bf),
            "Wk_r": np.ascontiguousarray(wk_t).astype(bf),
            "Wc_r": np.ascontiguousarray(wc_t).astype(bf),
            "bqk": np.ascontiguousarray(bqk),
        })
    return in_maps


def kernel(k, q, v, Wk, bk, Wq, bq, Wc, bc, _trace=False, _trace_cores=None):
    bc = np.asarray(bc, dtype=np.float32)
    nc = build_nc()
    in_maps = make_in_maps(k, q, v, Wk, bk, Wq, bq, Wc, bc)
    res = run_bass_kernel_spmd(
        nc, in_maps, core_ids=list(range(N_CORES)),
        trace=_trace, trace_cores=_trace_cores,
    )
    outs = [res.results[c]["out"].astype(np.float32) for c in range(N_CORES)]
    full = np.stack([
        outs[0] + outs[1] + outs[2] + outs[3],
        outs[4] + outs[5] + outs[6] + outs[7],
    ]) + bc[None, None, :]
    kernel.last_result = res
    return full.astype(np.float32)


# revision 18
# speedup vs baseline: 1.7704x; 1.0161x over previous
"""Trainium2 Bass kernel for nn_CrossAttention (B=2, T=2048, D=1024, H=16, hd=64).

Sharding: 32 (batch, head) units over 8 cores -> each core handles 1 batch and
4 contiguous heads (core c: batch c//4, heads (c%4)*4 .. +4), grouped as two
head-pairs j in {0,1}.  Host sums the 4 partial c_proj outputs per batch and
adds bc.

Per-core dataflow (bf16 operands, D-on-partitions activations), organized as
one long ScalarE exp stream that everything else hides behind:
  qpT/kpT [128, 2, 2048] bf16 = W.T @ xT (+b), K=1024 accumulated in PSUM.
  Attention runs per (pair j, q-chunk 512) at kv-tile grain: the two heads'
  K=64 score matmuls go to PE row-groups 0/64 concurrently into a double-
  buffered [128, 2, 512] f32 S tile; one exp (free-size 1024) -> es bf16;
  av matmuls ([v_h | ones].T @ es) accumulate y+colsum into [65, 2, 512]
  f32, lagging exp by LAG kv-steps.  Each unit's trailing avs are spread
  over the next unit's first steps; y is evacuated to SBUF early so the
  normalize chain (reciprocal_approx_fast -> gpsimd partition_broadcast ->
  multiply) never blocks PSUM reuse.  PSUM: S 2x2 banks + y 2 banks + a
  2-bank "aux" slot on which projection groups and c_proj t-tiles run as
  small "filler" bundles inside the attention steps, so only 3 projection
  groups precede the exp stream and only the last c_proj tiles follow it.
"""

import sys

sys.path.insert(0, "/opt/trn_rl_repo")

from collections import deque

import numpy as np
import ml_dtypes

import concourse.bacc as bacc
import concourse.bass as bass
import concourse.mybir as mybir
import concourse.tile as tile
from concourse.bass_utils import run_bass_kernel_spmd

F32 = mybir.dt.float32
BF16 = mybir.dt.bfloat16

T = 2048          # sequence length (q and kv)
D = 1024          # model dim
HL = 4            # heads per core
HD = 64           # head dim
DH = HL * HD      # 256 local projected dim
P = 128
QC = 512          # q chunk for attention (y/norm granularity)
NU = T // QC      # 4 q-chunks
NKV = T // P      # 16 kv tiles
SCALE = 1.0 / 8.0  # 1/sqrt(64)
LAG = 10          # kv-steps by which av trails exp

N_CORES = 8

_cache = {}


def build_nc():
    if "nc" in _cache:
        return _cache["nc"]
    nc = bacc.Bacc(
        "TRN2",
        target_bir_lowering=False,
        debug=False,
        num_devices=N_CORES,
    )

    qT = nc.declare_dram_parameter("qT", [D, T], BF16, isOutput=False)
    kT = nc.declare_dram_parameter("kT", [D, T], BF16, isOutput=False)
    vext = nc.declare_dram_parameter(
        "vext", [P, HL, NKV, HD + 1], BF16, isOutput=False
    )
    Wq_r = nc.declare_dram_parameter("Wq_r", [P, 8, DH], BF16, isOutput=False)
    Wk_r = nc.declare_dram_parameter("Wk_r", [P, 8, DH], BF16, isOutput=False)
    Wc_r = nc.declare_dram_parameter("Wc_r", [P, 2, D], BF16, isOutput=False)
    bqk = nc.declare_dram_parameter("bqk", [P, 4], F32, isOutput=False)
    out = nc.declare_dram_parameter("out", [T, D], BF16, isOutput=True)

    with tile.TileContext(nc) as tc:
        with (
            tc.tile_pool(name="wpool", bufs=1) as wpool,
            tc.tile_pool(name="xpool", bufs=16) as xpool,
            tc.tile_pool(name="projsb", bufs=1) as projsb,
            tc.tile_pool(name="espool", bufs=12) as espool,
            tc.tile_pool(name="npool", bufs=2) as npool,
            tc.tile_pool(name="opool", bufs=3) as opool,
            tc.tile_pool(name="ps", bufs=1, space="PSUM") as ps,
        ):
            # ---- weights / constants (host pre-packed, contiguous DMA) ----
            bias_sb = wpool.tile([P, 4], F32, name="bias_sb")  # [bq0,bq1,bk0,bk1]
            nc.sync.dma_start(bias_sb[:], bqk.ap())
            wk_sb = wpool.tile([P, 8, DH], BF16, name="wk_sb")
            nc.sync.dma_start(wk_sb[:], Wk_r.ap())
            wq_sb = wpool.tile([P, 8, DH], BF16, name="wq_sb")
            nc.sync.dma_start(wq_sb[:], Wq_r.ap())
            wc_sb = wpool.tile([P, 2, D], BF16, name="wc_sb")
            ve_sb = wpool.tile([P, HL, NKV, HD + 1], BF16, name="ve_sb")

            # preload the exp activation table during the DMA ramp
            warm_sb = wpool.tile([P, 4], F32, name="warm_sb")
            nc.scalar.activation(
                warm_sb[:], bias_sb[:], mybir.ActivationFunctionType.Exp,
                scale=0.0,
            )

            kpT = projsb.tile([P, 2, T], BF16, name="kpT")
            qpT = projsb.tile([P, 2, T], BF16, name="qpT")
            yallT = projsb.tile([P, 2, T], BF16, name="yallT")

            # ---- input streams, split by column half and ordered so each
            # projection group's operands arrive just-in-time:
            # k cols 0:1024 -> k cols 1024:2048 -> q cols 0:1024 -> rest
            xts = {}

            def xload(nm, xd, tcp):
                for i in range(8):
                    xt = xpool.tile([P, 1024], BF16, tag=f"x{tcp}",
                                    bufs=16, name=f"x{nm}{i}{tcp}")
                    nc.sync.dma_start(
                        xt[:],
                        xd.ap()[i * P:(i + 1) * P,
                                tcp * 1024:(tcp + 1) * 1024],
                    )
                    xts[nm, i, tcp] = xt

            xload("k", kT, 0)
            xload("k", kT, 1)
            xload("q", qT, 0)
            nc.sync.dma_start(ve_sb[:], vext.ap())
            nc.sync.dma_start(wc_sb[:], Wc_r.ap())
            xload("q", qT, 1)

            projw = {"k": (wk_sb, 2, kpT), "q": (wq_sb, 0, qpT)}

            def pgroup_closures(nm, j, tcp, tag="aux", bufs=1):
                """One projection PSUM group = 16 accumulating matmuls +
                bias evac on the aux slot, chopped into <=3-matmul filler
                closures.  Lifecycles on aux are strictly sequential."""
                w_sb, bias_col0, xpT = projw[nm]
                st = {}
                mms = [(i, t2) for i in range(8) for t2 in range(2)]

                def emit(lo, hi):
                    def go():
                        if "g" not in st:
                            st["g"] = ps.tile([P, 2, 512], F32, tag=tag,
                                              bufs=bufs,
                                              name=f"pg{nm}{j}{tcp}")
                        for i, t2 in mms[lo:hi]:
                            nc.tensor.matmul(
                                st["g"][:, t2, :],
                                w_sb[:, i, j * P:(j + 1) * P],
                                xts[nm, i, tcp][:, t2 * 512:(t2 + 1) * 512],
                                start=(i == 0),
                                stop=(i == 7),
                            )
                    return go

                def evac():
                    nc.vector.tensor_tensor(
                        xpT[:, j, tcp * 1024:(tcp + 1) * 1024],
                        st["g"].rearrange("p a b -> p (a b)"),
                        bias_sb[:, bias_col0 + j:bias_col0 + j + 1]
                        .to_broadcast((P, 1024)),
                        mybir.AluOpType.add,
                    )

                cs = [emit(lo, min(lo + 3, 16)) for lo in range(0, 16, 3)]
                cs.append(evac)
                return cs

            def cp_closure(tt):
                """One c_proj t-tile on the aux slot: 4 matmuls (K=256 over
                j) + evac + DMA, as a single small filler closure."""
                def go():
                    cp = ps.tile([P, 2, 512], F32, tag="aux", name=f"cp{tt}")
                    for j in range(2):
                        for nch in range(2):
                            nc.tensor.matmul(
                                cp[:, nch, :],
                                yallT[:, j, tt * P:(tt + 1) * P],
                                wc_sb[:, j, nch * 512:(nch + 1) * 512],
                                start=(j == 0),
                                stop=(j == 1),
                            )
                    o_sb = opool.tile([P, D], BF16, tag="osb", name=f"o{tt}")
                    nc.vector.tensor_copy(
                        o_sb[:], cp.rearrange("p a b -> p (a b)")
                    )
                    nc.sync.dma_start(
                        out.ap()[tt * P:(tt + 1) * P, :], o_sb[:]
                    )
                return go

            # inline projection prologue: what attention units (0,0)/(1,0)
            # need first (q j0 tcp0 covers q-chunks 0 and 1).  The k groups
            # run on the two "s" slots (still free) so they overlap.
            for cl in (pgroup_closures("k", 0, 0, tag="s", bufs=2)
                       + pgroup_closures("k", 0, 1, tag="s", bufs=2)
                       + pgroup_closures("q", 0, 0)):
                cl()

            filler = deque()
            for nm, j, tcp in (("k", 1, 0), ("q", 1, 0), ("k", 1, 1),
                               ("q", 0, 1), ("q", 1, 1)):
                filler.extend(pgroup_closures(nm, j, tcp))

            # ---- attention units ----
            def attn_unit(j, c):
                q0 = c * QC
                st = {}
                es_tiles = {}

                def step_mm(kk):
                    s_ps = ps.tile([P, 2, 512], F32, tag="s", bufs=2,
                                   name=f"s{j}{c}{kk}")
                    for s in range(2):  # head slot: partitions s*64..s*64+64
                        p0 = s * 64
                        nc.tensor.matmul(
                            s_ps[:, s, :],
                            kpT[p0:p0 + HD, j, kk * P:(kk + 1) * P],
                            qpT[p0:p0 + HD, j, q0:q0 + QC],
                            start=True,
                            stop=True,
                        )
                    es = espool.tile([P, 2, QC], BF16, tag="es",
                                     name=f"e{j}{c}{kk}")
                    nc.scalar.activation(
                        es[:], s_ps[:], mybir.ActivationFunctionType.Exp,
                        scale=SCALE,
                    )
                    es_tiles[kk] = es

                def av_mm(kk):
                    if "y" not in st:
                        st["y"] = ps.tile([P, 2, QC], F32, tag="y",
                                          name=f"y{j}{c}")
                    for s in range(2):
                        h = 2 * j + s
                        nc.tensor.matmul(
                            st["y"][0:HD + 1, s, :],
                            ve_sb[:, h, kk, :],
                            es_tiles[kk][:, s, :],
                            start=(kk == 0),
                            stop=(kk == NKV - 1),
                        )
                    del es_tiles[kk]

                def norm():
                    # evacuate y+colsum to SBUF first (frees the PSUM slot),
                    # then normalize off-PSUM.
                    y_ps = st["y"]
                    colsum2 = npool.tile([1, 2, QC], F32, tag="colsum",
                                         name=f"cs{j}{c}")
                    nc.vector.tensor_copy(colsum2[:], y_ps[HD:HD + 1, :, :])
                    yev = npool.tile([HD, 2, QC], F32, tag="yev",
                                     name=f"ye{j}{c}")
                    nc.vector.tensor_copy(yev[:], y_ps[0:HD, :, :])
                    recip2 = npool.tile([1, 2, QC], F32, tag="recip",
                                        name=f"rc{j}{c}")
                    nc.vector.reciprocal_approx_fast(
                        out=recip2[:], in_=colsum2[:]
                    )
                    bcast2 = npool.tile([HD, 2, QC], F32, tag="bcast",
                                        name=f"bc{j}{c}")
                    nc.gpsimd.partition_broadcast(bcast2[:], recip2[:])
                    nc.vector.tensor_tensor(
                        yallT[0:HD, j, q0:q0 + QC],
                        yev[:, 0, :], bcast2[:, 0, :],
                        mybir.AluOpType.mult,
                    )
                    yn = npool.tile([HD, QC], BF16, tag="yn", name=f"yn{j}{c}")
                    nc.vector.tensor_tensor(
                        yn[:], yev[:, 1, :], bcast2[:, 1, :],
                        mybir.AluOpType.mult,
                    )
                    nc.sync.dma_start(
                        yallT[64:64 + HD, j, q0:q0 + QC], yn[:]
                    )

                return step_mm, av_mm, norm

            units = [(j, c) for c in range(NU) for j in range(2)]
            prev = None
            for j, c in units:
                step_mm, av_mm, norm = attn_unit(j, c)
                for kk in range(NKV):
                    step_mm(kk)
                    if prev is not None:
                        # spread the previous unit's 10 trailing avs over
                        # kk 0..8 (2 on kk=0), then its normalize at kk=8 —
                        # its y-slot is free well before our av(0) at kk=LAG
                        if kk == 0:
                            prev["avs"].popleft()()
                        if kk <= 8 and prev["avs"]:
                            prev["avs"].popleft()()
                        if kk == 8:
                            prev["norm"]()
                            if j == 0 and c >= 1:
                                # q-chunk c-1 rows of yallT complete
                                for tt in range(4 * (c - 1), 4 * c):
                                    filler.append(cp_closure(tt))
                    if kk >= LAG:
                        av_mm(kk - LAG)
                    # pops start at kk=1 so every filler group's last closure
                    # is emitted strictly before its first consumer step
                    if kk >= 1 and filler:
                        filler.popleft()()
                prev = {
                    "avs": deque(
                        (lambda kk2=kk2, f=av_mm: f(kk2))
                        for kk2 in range(NKV - LAG, NKV)
                    ),
                    "norm": norm,
                }
            while prev["avs"]:
                prev["avs"].popleft()()
            prev["norm"]()
            for tt in range(4 * (NU - 1), 4 * NU):
                filler.append(cp_closure(tt))
            while filler:
                filler.popleft()()

    nc.compile()
    _cache["nc"] = nc
    return nc


def make_in_maps(k, q, v, Wk, bk, Wq, bq, Wc, bc):
    bf = ml_dtypes.bfloat16
    k = np.asarray(k, dtype=np.float32)
    q = np.asarray(q, dtype=np.float32)
    v = np.asarray(v, dtype=np.float32)
    Wk = np.asarray(Wk, dtype=np.float32)
    Wq = np.asarray(Wq, dtype=np.float32)
    Wc = np.asarray(Wc, dtype=np.float32)
    bk = np.asarray(bk, dtype=np.float32)
    bq = np.asarray(bq, dtype=np.float32)
    in_maps = []
    for cidx in range(N_CORES):
        b = cidx // 4
        h0 = (cidx % 4) * HL
        sl = slice(h0 * HD, h0 * HD + DH)
        bq_t = np.ascontiguousarray(bq[sl].reshape(2, P).T)  # [128, 2]
        bk_t = np.ascontiguousarray(bk[sl].reshape(2, P).T)
        bqk = np.concatenate([bq_t, bk_t], axis=1)           # [128, 4]
        # vext [P, HL, NKV, HD+1]: [p, h, m, d] = v[m*128+p, sl][h*64+d],
        # ones at d=64 (colsum row for the av matmul)
        vsl = v[b][:, sl]                                    # [T, 256]
        ve = np.ones((P, HL, NKV, HD + 1), dtype=np.float32)
        ve[:, :, :, 0:HD] = (
            vsl.reshape(NKV, P, HL, HD).transpose(1, 2, 0, 3)
        )
        # Wq_r [128, 8, 256]: [p, i, m] = Wq[sl,:].T[i*128+p, m]
        wq_t = Wq[sl, :].T.reshape(8, P, DH).transpose(1, 0, 2)
        wk_t = Wk[sl, :].T.reshape(8, P, DH).transpose(1, 0, 2)
        wc_t = Wc[:, sl].T.reshape(2, P, D).transpose(1, 0, 2)
        in_maps.append({
            "qT": np.ascontiguousarray(q[b].T).astype(bf),
            "kT": np.ascontiguousarray(k[b].T).astype(bf),
            "vext": np.ascontiguousarray(ve).astype(bf),
            "Wq_r": np.ascontiguousarray(wq_t).astype(bf),
            "Wk_r": np.ascontiguousarray(wk_t).astype(bf),
            "Wc_r": np.ascontiguousarray(wc_t).astype(bf),
            "bqk": np.ascontiguousarray(bqk),
        })
    return in_maps


def kernel(k, q, v, Wk, bk, Wq, bq, Wc, bc, _trace=False, _trace_cores=None):
    bc = np.asarray(bc, dtype=np.float32)
    nc = build_nc()
    in_maps = make_in_maps(k, q, v, Wk, bk, Wq, bq, Wc, bc)
    res = run_bass_kernel_spmd(
        nc, in_maps, core_ids=list(range(N_CORES)),
        trace=_trace, trace_cores=_trace_cores,
    )
    outs = [res.results[c]["out"].astype(np.float32) for c in range(N_CORES)]
    full = np.stack([
        outs[0] + outs[1] + outs[2] + outs[3],
        outs[4] + outs[5] + outs[6] + outs[7],
    ]) + bc[None, None, :]
    kernel.last_result = res
    return full.astype(np.float32)


# revision 22
# speedup vs baseline: 1.7947x; 1.0137x over previous
"""Trainium2 Bass kernel for nn_CrossAttention (B=2, T=2048, D=1024, H=16, hd=64).

Sharding: 32 (batch, head) units over 8 cores -> each core handles 1 batch and
4 contiguous heads (core c: batch c//4, heads (c%4)*4 .. +4), grouped as two
head-pairs j in {0,1}.  Host sums the 4 partial c_proj outputs per batch and
adds bc.

Per-core dataflow (bf16 operands, D-on-partitions activations), organized as
one long ScalarE exp stream that everything else hides behind:
  qpT/kpT [128, 2, 2048] bf16 = W.T @ xT (+b), K=1024 accumulated in PSUM.
  Attention runs per (pair j, q-chunk 512) at kv-tile grain: the two heads'
  K=64 score matmuls go to PE row-groups 0/64 concurrently into a double-
  buffered [128, 2, 512] f32 S tile; one exp (free-size 1024) -> es bf16;
  av matmuls ([v_h | ones].T @ es) accumulate y+colsum into [65, 2, 512]
  f32, lagging exp by LAG kv-steps.  Each unit's trailing avs are spread
  over the next unit's first steps; y is evacuated to SBUF early so the
  normalize chain (reciprocal_approx_fast -> gpsimd partition_broadcast ->
  multiply) never blocks PSUM reuse.  PSUM: S 2x2 banks + y 2 banks + a
  2-bank "aux" slot on which projection groups and c_proj t-tiles run as
  small "filler" bundles inside the attention steps, so only 3 projection
  groups precede the exp stream and only the last c_proj tiles follow it.
"""

import sys

sys.path.insert(0, "/opt/trn_rl_repo")

from collections import deque

import numpy as np
import ml_dtypes

import concourse.bacc as bacc
import concourse.bass as bass
import concourse.mybir as mybir
import concourse.tile as tile
from concourse.bass_utils import run_bass_kernel_spmd

F32 = mybir.dt.float32
BF16 = mybir.dt.bfloat16

T = 2048          # sequence length (q and kv)
D = 1024          # model dim
HL = 4            # heads per core
HD = 64           # head dim
DH = HL * HD      # 256 local projected dim
P = 128
QC = 512          # q chunk for attention (y/norm granularity)
NU = T // QC      # 4 q-chunks
NKV = T // P      # 16 kv tiles
SCALE = 1.0 / 8.0  # 1/sqrt(64)
LAG = 10          # kv-steps by which av trails exp

N_CORES = 8

_cache = {}


def build_nc():
    if "nc" in _cache:
        return _cache["nc"]
    nc = bacc.Bacc(
        "TRN2",
        target_bir_lowering=False,
        debug=False,
        num_devices=N_CORES,
    )

    qT = nc.declare_dram_parameter("qT", [D, T], BF16, isOutput=False)
    kT = nc.declare_dram_parameter("kT", [D, T], BF16, isOutput=False)
    vext = nc.declare_dram_parameter(
        "vext", [P, HL, NKV, HD + 1], BF16, isOutput=False
    )
    Wq_r = nc.declare_dram_parameter("Wq_r", [P, 8, DH], BF16, isOutput=False)
    Wk_r = nc.declare_dram_parameter("Wk_r", [P, 8, DH], BF16, isOutput=False)
    Wc_r = nc.declare_dram_parameter("Wc_r", [P, 2, D], BF16, isOutput=False)
    bqk = nc.declare_dram_parameter("bqk", [P, 4], F32, isOutput=False)
    out = nc.declare_dram_parameter("out", [T, D], BF16, isOutput=True)

    with tile.TileContext(nc) as tc:
        with (
            tc.tile_pool(name="wpool", bufs=1) as wpool,
            tc.tile_pool(name="xpool", bufs=16) as xpool,
            tc.tile_pool(name="projsb", bufs=1) as projsb,
            tc.tile_pool(name="espool", bufs=12) as espool,
            tc.tile_pool(name="npool", bufs=2) as npool,
            tc.tile_pool(name="opool", bufs=3) as opool,
            tc.tile_pool(name="ps", bufs=1, space="PSUM") as ps,
        ):
            # ---- weights / constants (host pre-packed, contiguous DMA) ----
            bias_sb = wpool.tile([P, 4], F32, name="bias_sb")  # [bq0,bq1,bk0,bk1]
            nc.sync.dma_start(bias_sb[:], bqk.ap())
            wk_sb = wpool.tile([P, 8, DH], BF16, name="wk_sb")
            nc.sync.dma_start(wk_sb[:], Wk_r.ap())
            wq_sb = wpool.tile([P, 8, DH], BF16, name="wq_sb")
            nc.sync.dma_start(wq_sb[:], Wq_r.ap())
            wc_sb = wpool.tile([P, 2, D], BF16, name="wc_sb")
            ve_sb = wpool.tile([P, HL, NKV, HD + 1], BF16, name="ve_sb")

            # preload the exp activation table during the DMA ramp
            warm_sb = wpool.tile([P, 4], F32, name="warm_sb")
            nc.scalar.activation(
                warm_sb[:], bias_sb[:], mybir.ActivationFunctionType.Exp,
                scale=0.0,
            )

            kpT = projsb.tile([P, 2, T], BF16, name="kpT")
            qpT = projsb.tile([P, 2, T], BF16, name="qpT")
            yallT = projsb.tile([P, 2, T], BF16, name="yallT")

            # ---- input streams, split by column half and ordered so each
            # projection group's operands arrive just-in-time:
            # k cols 0:1024 -> k cols 1024:2048 -> q cols 0:1024 -> rest
            xts = {}

            def xload(nm, xd, tcp):
                for i in range(8):
                    xt = xpool.tile([P, 1024], BF16, tag=f"x{tcp}",
                                    bufs=16, name=f"x{nm}{i}{tcp}")
                    nc.sync.dma_start(
                        xt[:],
                        xd.ap()[i * P:(i + 1) * P,
                                tcp * 1024:(tcp + 1) * 1024],
                    )
                    xts[nm, i, tcp] = xt

            xload("k", kT, 0)
            xload("q", qT, 0)
            nc.sync.dma_start(ve_sb[:], vext.ap())
            nc.sync.dma_start(wc_sb[:], Wc_r.ap())
            xload("k", kT, 1)
            xload("q", qT, 1)

            projw = {"k": (wk_sb, 2, kpT), "q": (wq_sb, 0, qpT)}

            def pgroup_closures(nm, j, tcp, tag="aux", bufs=1):
                """One projection PSUM group = 16 accumulating matmuls +
                bias evac on the aux slot, chopped into <=3-matmul filler
                closures.  Lifecycles on aux are strictly sequential."""
                w_sb, bias_col0, xpT = projw[nm]
                st = {}
                mms = [(i, t2) for i in range(8) for t2 in range(2)]

                def emit(lo, hi):
                    def go():
                        if "g" not in st:
                            st["g"] = ps.tile([P, 2, 512], F32, tag=tag,
                                              bufs=bufs,
                                              name=f"pg{nm}{j}{tcp}")
                        for i, t2 in mms[lo:hi]:
                            nc.tensor.matmul(
                                st["g"][:, t2, :],
                                w_sb[:, i, j * P:(j + 1) * P],
                                xts[nm, i, tcp][:, t2 * 512:(t2 + 1) * 512],
                                start=(i == 0),
                                stop=(i == 7),
                            )
                    return go

                def evac():
                    nc.vector.tensor_tensor(
                        xpT[:, j, tcp * 1024:(tcp + 1) * 1024],
                        st["g"].rearrange("p a b -> p (a b)"),
                        bias_sb[:, bias_col0 + j:bias_col0 + j + 1]
                        .to_broadcast((P, 1024)),
                        mybir.AluOpType.add,
                    )

                cs = [emit(lo, min(lo + 3, 16)) for lo in range(0, 16, 3)]
                cs.append(evac)
                return cs

            def cp_closure(tt):
                """One c_proj t-tile on the aux slot: 4 matmuls (K=256 over
                j) + evac + DMA, as a single small filler closure."""
                def go():
                    cp = ps.tile([P, 2, 512], F32, tag="aux", name=f"cp{tt}")
                    for j in range(2):
                        for nch in range(2):
                            nc.tensor.matmul(
                                cp[:, nch, :],
                                yallT[:, j, tt * P:(tt + 1) * P],
                                wc_sb[:, j, nch * 512:(nch + 1) * 512],
                                start=(j == 0),
                                stop=(j == 1),
                            )
                    o_sb = opool.tile([P, D], BF16, tag="osb", name=f"o{tt}")
                    nc.vector.tensor_copy(
                        o_sb[:], cp.rearrange("p a b -> p (a b)")
                    )
                    nc.sync.dma_start(
                        out.ap()[tt * P:(tt + 1) * P, :], o_sb[:]
                    )
                return go

            # inline projection prologue: only what attention unit (0,0)
            # kk 0..7 needs (k j0 cols 0:1024, q j0 cols 0:1024 which covers
            # q-chunks 0 and 1).  The k group runs on the "s" slots (still
            # free) so the q group overlaps it on aux.
            for cl in (pgroup_closures("k", 0, 0, tag="s", bufs=2)
                       + pgroup_closures("q", 0, 0)):
                cl()

            # remaining projection groups stream in as filler, ordered by
            # first-consumer deadline (k01 by unit0 kk=8; k10/q10 by unit1;
            # k11 by unit1 kk=8; q01 by unit4; q11 by unit5)
            filler = deque()
            for nm, j, tcp in (("k", 0, 1), ("k", 1, 0), ("q", 1, 0),
                               ("k", 1, 1), ("q", 0, 1), ("q", 1, 1)):
                filler.extend(pgroup_closures(nm, j, tcp))

            # ---- attention units ----
            def attn_unit(j, c):
                q0 = c * QC
                st = {}
                es_tiles = {}

                def step_mm(kk):
                    s_ps = ps.tile([P, 2, 512], F32, tag="s", bufs=2,
                                   name=f"s{j}{c}{kk}")
                    for s in range(2):  # head slot: partitions s*64..s*64+64
                        p0 = s * 64
                        nc.tensor.matmul(
                            s_ps[:, s, :],
                            kpT[p0:p0 + HD, j, kk * P:(kk + 1) * P],
                            qpT[p0:p0 + HD, j, q0:q0 + QC],
                            start=True,
                            stop=True,
                        )
                    es = espool.tile([P, 2, QC], BF16, tag="es",
                                     name=f"e{j}{c}{kk}")
                    nc.scalar.activation(
                        es[:], s_ps[:], mybir.ActivationFunctionType.Exp,
                        scale=SCALE,
                    )
                    es_tiles[kk] = es

                def av_mm(kk):
                    if "y" not in st:
                        st["y"] = ps.tile([P, 2, QC], F32, tag="y",
                                          name=f"y{j}{c}")
                    for s in range(2):
                        h = 2 * j + s
                        nc.tensor.matmul(
                            st["y"][0:HD + 1, s, :],
                            ve_sb[:, h, kk, :],
                            es_tiles[kk][:, s, :],
                            start=(kk == 0),
                            stop=(kk == NKV - 1),
                        )
                    del es_tiles[kk]

                def norm():
                    # evacuate y+colsum to SBUF first (frees the PSUM slot),
                    # then normalize off-PSUM.
                    y_ps = st["y"]
                    colsum2 = npool.tile([1, 2, QC], F32, tag="colsum",
                                         name=f"cs{j}{c}")
                    nc.vector.tensor_copy(colsum2[:], y_ps[HD:HD + 1, :, :])
                    yev = npool.tile([HD, 2, QC], F32, tag="yev",
                                     name=f"ye{j}{c}")
                    nc.vector.tensor_copy(yev[:], y_ps[0:HD, :, :])
                    recip2 = npool.tile([1, 2, QC], F32, tag="recip",
                                        name=f"rc{j}{c}")
                    nc.vector.reciprocal_approx_fast(
                        out=recip2[:], in_=colsum2[:]
                    )
                    bcast2 = npool.tile([HD, 2, QC], F32, tag="bcast",
                                        name=f"bc{j}{c}")
                    nc.gpsimd.partition_broadcast(bcast2[:], recip2[:])
                    nc.vector.tensor_tensor(
                        yallT[0:HD, j, q0:q0 + QC],
                        yev[:, 0, :], bcast2[:, 0, :],
                        mybir.AluOpType.mult,
                    )
                    yn = npool.tile([HD, QC], BF16, tag="yn", name=f"yn{j}{c}")
                    nc.vector.tensor_tensor(
                        yn[:], yev[:, 1, :], bcast2[:, 1, :],
                        mybir.AluOpType.mult,
                    )
                    nc.sync.dma_start(
                        yallT[64:64 + HD, j, q0:q0 + QC], yn[:]
                    )

                return step_mm, av_mm, norm

            units = [(j, c) for c in range(NU) for j in range(2)]
            prev = None
            for u_idx, (j, c) in enumerate(units):
                step_mm, av_mm, norm = attn_unit(j, c)
                for kk in range(NKV):
                    step_mm(kk)
                    if prev is not None:
                        # spread the previous unit's 10 trailing avs over
                        # kk 0..8 (2 on kk=0), then its normalize at kk=8 —
                        # its y-slot is free well before our av(0) at kk=LAG
                        if kk == 0:
                            prev["avs"].popleft()()
                        if kk <= 8 and prev["avs"]:
                            prev["avs"].popleft()()
                        if kk == 8:
                            prev["norm"]()
                            if j == 0 and c >= 1:
                                # q-chunk c-1 rows of yallT complete
                                for tt in range(4 * (c - 1), 4 * c):
                                    filler.append(cp_closure(tt))
                    if kk >= LAG:
                        av_mm(kk - LAG)
                    # pops start at kk=1 so every filler group's last closure
                    # is emitted strictly before its first consumer step;
                    # unit 0 has no trailing avs, so it pops double after
                    # kk=8 to retire k10/q10 before unit 1 begins
                    if kk >= 1 and filler:
                        filler.popleft()()
                    if u_idx == 0 and kk >= 8 and filler:
                        filler.popleft()()
                prev = {
                    "avs": deque(
                        (lambda kk2=kk2, f=av_mm: f(kk2))
                        for kk2 in range(NKV - LAG, NKV)
                    ),
                    "norm": norm,
                }
            while prev["avs"]:
                prev["avs"].popleft()()
            prev["norm"]()
            for tt in range(4 * (NU - 1), 4 * NU):
                filler.append(cp_closure(tt))
            while filler:
                filler.popleft()()

    nc.compile()
    _cache["nc"] = nc
    return nc


def make_in_maps(k, q, v, Wk, bk, Wq, bq, Wc, bc):
    bf = ml_dtypes.bfloat16
    k = np.asarray(k, dtype=np.float32)
    q = np.asarray(q, dtype=np.float32)
    v = np.asarray(v, dtype=np.float32)
    Wk = np.asarray(Wk, dtype=np.float32)
    Wq = np.asarray(Wq, dtype=np.float32)
    Wc = np.asarray(Wc, dtype=np.float32)
    bk = np.asarray(bk, dtype=np.float32)
    bq = np.asarray(bq, dtype=np.float32)
    in_maps = []
    for cidx in range(N_CORES):
        b = cidx // 4
        h0 = (cidx % 4) * HL
        sl = slice(h0 * HD, h0 * HD + DH)
        bq_t = np.ascontiguousarray(bq[sl].reshape(2, P).T)  # [128, 2]
        bk_t = np.ascontiguousarray(bk[sl].reshape(2, P).T)
        bqk = np.concatenate([bq_t, bk_t], axis=1)           # [128, 4]
        # vext [P, HL, NKV, HD+1]: [p, h, m, d] = v[m*128+p, sl][h*64+d],
        # ones at d=64 (colsum row for the av matmul)
        vsl = v[b][:, sl]                                    # [T, 256]
        ve = np.ones((P, HL, NKV, HD + 1), dtype=np.float32)
        ve[:, :, :, 0:HD] = (
            vsl.reshape(NKV, P, HL, HD).transpose(1, 2, 0, 3)
        )
        # Wq_r [128, 8, 256]: [p, i, m] = Wq[sl,:].T[i*128+p, m]
        wq_t = Wq[sl, :].T.reshape(8, P, DH).transpose(1, 0, 2)
        wk_t = Wk[sl, :].T.reshape(8, P, DH).transpose(1, 0, 2)
        wc_t = Wc[:, sl].T.reshape(2, P, D).transpose(1, 0, 2)
        in_maps.append({
            "qT": np.ascontiguousarray(q[b].T).astype(bf),
            "kT": np.ascontiguousarray(k[b].T).astype(bf),
            "vext": np.ascontiguousarray(ve).astype(bf),
            "Wq_r": np.ascontiguousarray(wq_t).astype(bf),
            "Wk_r": np.ascontiguousarray(wk_t).astype(bf),
            "Wc_r": np.ascontiguousarray(wc_t).astype(bf),
            "bqk": np.ascontiguousarray(bqk),
        })
    return in_maps


def kernel(k, q, v, Wk, bk, Wq, bq, Wc, bc, _trace=False, _trace_cores=None):
    bc = np.asarray(bc, dtype=np.float32)
    nc = build_nc()
    in_maps = make_in_maps(k, q, v, Wk, bk, Wq, bq, Wc, bc)
    res = run_bass_kernel_spmd(
        nc, in_maps, core_ids=list(range(N_CORES)),
        trace=_trace, trace_cores=_trace_cores,
    )
    outs = [res.results[c]["out"].astype(np.float32) for c in range(N_CORES)]
    full = np.stack([
        outs[0] + outs[1] + outs[2] + outs[3],
        outs[4] + outs[5] + outs[6] + outs[7],
    ]) + bc[None, None, :]
    kernel.last_result = res
    return full.astype(np.float32)
